# revision 76
# baseline (speedup 1.0000x reference)
"""Trainium2 Bass kernel for nn_MixGNN (TransformerConv + 3x SAGEConv + BN + gated residual).

Strategy (8 NeuronCores, dst-node sharding):
  - Pad N 10000 -> 10240; core r owns 1280 dst nodes = 10 tiles of 128.
  - Host prep: degree-balanced node renumbering (each dst-tile gets ~E/80
    edges, shrinking the padded chunk count S), edges sorted by dst and
    bucketed per tile, wrapped int16 gather indices, per-chunk local-dst
    columns, 1/deg, bf16-packed weights with BN gamma/gate folded into
    Wl/Wr columns and an extra oma*I block for the PE-side residual.
  - Device per layer: dense matmuls on PE with bf16 operands (1 cycle/row);
    per-edge work via dma_gather of source rows (512B descriptors) +
    indicator matmuls (Ind[e,n] = (dst_e==n) built by DVE is_equal).
    Transformer: scores KgT.T @ qT from a transposed bf16 gather, batched
    exp() over 4 score chunks per Act call, w_b = Ind*exp via one fused DVE
    op, agg + exp-sum accumulated in one PSUM group; epilogue adds x@Ws with
    the bias via a rank-1 ones-row matmul. SAGE: TRANSPOSED scatter
    (paggT_j[d_j,dst] in two distinct PSUM banks), pzA = sumT^T @ Wl' with
    1/deg applied after (commutes), and pzB = h@Wr' + bias + oma*h computed
    entirely on PE (bias via ones-row, residual via oma*I against hT) and
    issued before the gathers so PE fills the wait; epilogue is 2 DVE ops +
    relu, with non-final h kept bf16 so relu output feeds the AllGather
    staging and the hT transposes directly.
  - Halo exchange: AllGather of each core's h shard (bf16) into a full table
    in shared DRAM before every aggregation.
Output: fp32 [10000, 256].
"""
import os
import sys
import time

import numpy as np

for _p in ("/opt/trn_rl_repo",):
    if _p not in sys.path:
        sys.path.insert(0, _p)

import ml_dtypes  # noqa: E402
import concourse.bacc as bacc  # noqa: E402
import concourse.mybir as mybir  # noqa: E402
import concourse.tile as tile  # noqa: E402
from concourse.bass_utils import run_bass_kernel_spmd  # noqa: E402

P = 128
D = 256
DJ = D // P           # 2 d-chunks of 128
NC = 8                # cores
L = 3                 # SAGE layers
BN_EPS = 1e-5

# dtype knobs for gathered tables (accuracy vs bandwidth)
V_BF16 = True         # v table + attention agg in bf16
H_BF16 = True         # h tables + SAGE agg in bf16

F32 = mybir.dt.float32
F32R = (mybir.dt.float32 if os.environ.get("KNOF32R") == "1"
        else mybir.dt.float32r)  # f32 bits, PE full-rate mode at >=256 free
BF16 = mybir.dt.bfloat16
I16 = mybir.dt.int16
# dense-path operand dtype: bf16 halves DMA + SBUF for x/W/hT/mT tables
DENSE_BF16 = os.environ.get("KDENSE", "bf16") == "bf16"
D_DT = BF16 if DENSE_BF16 else F32R
D_NP = ml_dtypes.bfloat16 if DENSE_BF16 else np.float32
V_DT = BF16 if V_BF16 else F32
H_DT = BF16 if H_BF16 else F32

_nc_cache = {}


def _wrap_idx(a):
    """[S*128] int array -> [128, S*8] int16 wrapped gather-index layout."""
    w16 = a.reshape(-1, 16).T.astype(np.int16)   # [16, S*8]
    return np.tile(w16, (8, 1))                  # replicate to 8 Q7 stripes


def build_nc(n_pad, sh, nt, S, scale, oma):
    stages = int(os.environ.get("KSTAGES", "5"))
    nocc = os.environ.get("KNOCC") == "1"
    ksm = int(os.environ.get("KSM", "10"))
    kgp = int(os.environ.get("KGP", "2"))
    kpsc = int(os.environ.get("KPSC", "3"))
    kptr = int(os.environ.get("KPTR", "1"))
    kpagg = int(os.environ.get("KPAGG", "2"))
    kpmm = int(os.environ.get("KPMM", "2"))
    khalf = int(os.environ.get("KHALF", "4"))  # gather splits per tile
    kabl = os.environ.get("KABL", "")
    key = (n_pad, sh, nt, S, round(scale, 9), round(oma, 9), V_BF16, H_BF16, stages,
           nocc, ksm, kgp, kpsc, kptr, kpagg, kpmm, khalf, kabl,
           os.environ.get("KHALFT"),
           os.environ.get("KKGT"), os.environ.get("KVG"),
           os.environ.get("KEXP"), os.environ.get("KPOOLB"),
           os.environ.get("KWBSTT"), os.environ.get("KGRAD"))
    if key in _nc_cache:
        return _nc_cache[key]

    ET = S * P  # padded edges per tile
    ndev = 1 if nocc else NC
    nc = bacc.Bacc("TRN2", target_bir_lowering=False, debug=False, num_devices=ndev)

    xt_in = nc.dram_tensor("xt_in", [P, DJ * sh], D_DT, kind="ExternalInput")
    wpack_in = nc.dram_tensor("wpack_in", [P, 11 * DJ * D], D_DT, kind="ExternalInput")
    vpack_in = nc.dram_tensor("vpack_in", [P, 9 * D + DJ], F32, kind="ExternalInput")
    idx_in = nc.dram_tensor("idx_in", [P, nt * S * 8], I16, kind="ExternalInput")
    dst_in = nc.dram_tensor("dst_in", [P, nt * S], F32, kind="ExternalInput")
    invdeg_in = nc.dram_tensor("invdeg_in", [P, nt], F32, kind="ExternalInput")
    out_dram = nc.dram_tensor("out", [sh, D], F32, kind="ExternalOutput")

    WQ, WK, WV, WS = 0, 1, 2, 3
    WL = [4, 6, 8]
    WR = [5, 7, 9]
    VBK, VBV, VBS = 0, 1, 2

    with tile.TileContext(nc) as tc:
        with (
            tc.tile_pool(name="cst", bufs=1) as cst,
            tc.tile_pool(name="sb", bufs=1) as sb,
            tc.tile_pool(name="g", bufs=kgp) as gp,
            tc.tile_pool(name="sm", bufs=ksm) as smp,
            tc.tile_pool(name="ps", bufs=2, space="PSUM") as ps,
            tc.tile_pool(name="dr", bufs=1, space="DRAM") as dr,
        ):
            # ---------------- constants / inputs to SBUF ----------------
            wp = cst.tile([P, 11 * DJ * D], D_DT)
            nc.sync.dma_start(out=wp[:], in_=wpack_in[:])
            vp = cst.tile([P, 9 * D + DJ], F32)
            nc.sync.dma_start(out=vp[:], in_=vpack_in[:])
            xt = cst.tile([P, DJ * sh], D_DT)
            for _xi in range(4):
                _c0 = _xi * (DJ * sh // 4)
                _c1 = (_xi + 1) * (DJ * sh // 4)
                nc.sync.dma_start(out=xt[:, _c0:_c1], in_=xt_in[:, _c0:_c1])
            dstc = cst.tile([P, nt * S], F32)
            nc.sync.dma_start(out=dstc[:], in_=dst_in[:])
            invd = cst.tile([P, nt], F32)
            nc.sync.dma_start(out=invd[:], in_=invdeg_in[:])
            idx_sb = cst.tile([P, nt * S * 8], I16)
            nc.sync.dma_start(out=idx_sb[:], in_=idx_in[:])

            iota_i = cst.tile([P, P], mybir.dt.int32)
            nc.gpsimd.iota(iota_i[:], pattern=[[1, P]], base=0, channel_multiplier=0)
            ones_v = cst.tile([P, 1], V_DT)
            nc.vector.memset(ones_v[:], 1.0)
            # identity for PE transposes: (iota_row == partition_idx)
            iota_part = cst.tile([P, 1], mybir.dt.int32)
            nc.gpsimd.iota(iota_part[:], pattern=[[1, 1]], base=0, channel_multiplier=1)
            iota_part_f = cst.tile([P, 1], F32)
            nc.vector.tensor_copy(out=iota_part_f[:], in_=iota_part[:])
            iota_b = cst.tile([P, P], BF16)
            nc.vector.tensor_copy(out=iota_b[:], in_=iota_i[:])
            ident_b = cst.tile([P, P], BF16)
            nc.vector.tensor_scalar(
                out=ident_b[:], in0=iota_b[:], scalar1=iota_part_f[:, :1],
                scalar2=None, op0=mybir.AluOpType.is_equal,
            )
            # rank-1 bias rows + oma*I blocks (bias/residual folded into PE)
            ones1 = cst.tile([1, P], BF16)
            nc.vector.memset(ones1[:], 1.0)
            brow = cst.tile([1, (L + 1) * D], BF16)
            nc.vector.tensor_copy(out=brow[0:1, :D], in_=vp[0:1, 2 * D:3 * D])
            for _i in range(L):
                nc.vector.tensor_copy(
                    out=brow[0:1, (_i + 1) * D:(_i + 2) * D],
                    in_=vp[0:1, (4 + 2 * _i) * D:(5 + 2 * _i) * D])
            omaI = wp[:, 10 * DJ * D:11 * DJ * D]  # host-packed oma*I blocks

            def wslice(w, j):
                return wp[:, (w * DJ + j) * D:(w * DJ + j + 1) * D]

            def vslice(k):
                return vp[:, k * D:(k + 1) * D]

            def xtile(j, t):
                return xt[:, j * sh + t * P: j * sh + (t + 1) * P]

            # ---------------- DRAM tables ----------------
            k_ag_in = dr.tile([sh, D], BF16)
            v_ag_in = dr.tile([sh, D], V_DT)
            k_full = dr.tile([n_pad, D], BF16, addr_space="Shared")
            v_full = dr.tile([n_pad, D], V_DT, addr_space="Shared")
            hag_in = [dr.tile([sh, D], H_DT, name=f"hag_in_{i}") for i in range(L)]
            h_full = [dr.tile([n_pad, D], H_DT, name=f"h_full_{i}", addr_space="Shared")
                      for i in range(L)]

            def allgather(in_t, out_t):
                if nocc:
                    nc.sync.dma_start(out=out_t[:sh], in_=in_t[:])
                else:
                    nc.gpsimd.collective_compute(
                        "AllGather", mybir.AluOpType.bypass,
                        replica_groups=[list(range(NC))],
                        ins=[in_t[:]], outs=[out_t[:]],
                    )

            # ---------------- stage 0: k,v shard tables + AG, then qT ----------------
            for t in range(nt):
                pk = ps.tile([P, D], F32, name="pk", tag="pmm", bufs=kpmm)
                for ji in range(DJ):
                    nc.tensor.matmul(pk[:], lhsT=xtile(ji, t), rhs=wslice(WK, ji),
                                     start=(ji == 0), stop=(ji == DJ - 1))
                k_sb = smp.tile([P, D], BF16, name="k_sb")
                nc.vector.tensor_tensor(out=k_sb[:], in0=pk[:], in1=vslice(VBK),
                                        op=mybir.AluOpType.add)
                nc.sync.dma_start(out=k_ag_in[t * P:(t + 1) * P, :], in_=k_sb[:])

                pv = ps.tile([P, D], F32, name="pv", tag="pmm", bufs=kpmm)
                for ji in range(DJ):
                    nc.tensor.matmul(pv[:], lhsT=xtile(ji, t), rhs=wslice(WV, ji),
                                     start=(ji == 0), stop=(ji == DJ - 1))
                v_sb = smp.tile([P, D], V_DT, name="v_sb")
                nc.vector.tensor_tensor(out=v_sb[:], in0=pv[:], in1=vslice(VBV),
                                        op=mybir.AluOpType.add)
                nc.sync.dma_start(out=v_ag_in[t * P:(t + 1) * P, :], in_=v_sb[:])

            allgather(k_ag_in, k_full)
            allgather(v_ag_in, v_full)

            qT = []
            for j in range(DJ):
                qTj = sb.tile([P, sh], BF16, name=f"qT_{j}")
                n0 = 0
                while n0 < sh:
                    nn = min(512, sh - n0)
                    pq = ps.tile([P, 512], F32, name="pq", tag="pmm", bufs=kpmm)
                    for ji in range(DJ):
                        nc.tensor.matmul(
                            pq[:, :nn],
                            lhsT=wslice(WQ, ji)[:, j * P:(j + 1) * P],
                            rhs=xt[:, ji * sh + n0: ji * sh + n0 + nn],
                            start=(ji == 0), stop=(ji == DJ - 1),
                        )
                    nc.vector.tensor_scalar(
                        out=qTj[:, n0:n0 + nn], in0=pq[:, :nn],
                        scalar1=vp[:, 9 * D + j: 9 * D + j + 1], scalar2=None,
                        op0=mybir.AluOpType.add,
                    )
                    n0 += nn
                qT.append(qTj)

            # shard-resident activations
            h_cur = sb.tile([P, nt * D], H_DT)
            h_nxt = sb.tile([P, nt * D], H_DT)
            hT_cur = sb.tile([P, DJ * sh], D_DT)
            hT_nxt = sb.tile([P, DJ * sh], D_DT)

            def agg_pass(layer, h_prev, hT_prev, h_out, hT_out):
                """layer -1: transformer (h_prev/hT_prev unused); 0..L-1: SAGE."""
                li = layer + 1  # h table index this pass WRITES (0 for transformer)
                kh = khalf if layer >= 0 else int(os.environ.get("KHALFT", "1"))
                splits = []  # (c0, c1) chunk ranges per gather piece
                base = (S + kh - 1) // kh
                c0 = 0
                while c0 < S:
                    splits.append((c0, min(S, c0 + base)))
                    c0 += base
                # graded head for tile 0: a small leading split lets chunk-0
                # consumers start right after the pass-gating AllGather
                kgrad = int(os.environ.get("KGRAD", "0"))
                splits0 = splits
                if kgrad and layer >= 0:
                    splits0 = [(0, kgrad)]
                    c0 = kgrad
                    while c0 < S:
                        splits0.append((c0, min(S, c0 + base)))
                        c0 += base
                for t in range(nt):
                    if layer < 0:
                        kgt = gp.tile([P, DJ, ET], BF16, name="kgt", tag="kgt",
                                      bufs=int(os.environ.get("KKGT", "2")))
                        vg = gp.tile([P, S, D], V_DT, name="vg", tag="vg",
                                     bufs=int(os.environ.get("KVG", "3")))
                        pzB = None
                    else:
                        kgt = None
                        vg = gp.tile([P, S, D], H_DT, name="hg", tag="vg",
                                     bufs=int(os.environ.get("KVG", "3")))
                        # root + bias + gated residual, all on PE — independent
                        # of the gather; issue first so PE fills the wait.
                        pzB = ps.tile([P, D], F32, name="pzB", tag="pmm", bufs=kpmm)
                        for j in range(DJ):
                            nc.tensor.matmul(
                                pzB[:],
                                lhsT=hT_prev[:, j * sh + t * P: j * sh + (t + 1) * P],
                                rhs=wslice(WR[layer], j),
                                start=(j == 0), stop=False)
                        nc.tensor.matmul(
                            pzB[:], lhsT=ones1[:],
                            rhs=brow[0:1, (layer + 1) * D:(layer + 2) * D],
                            start=False, stop=False)
                        for j in range(DJ):
                            nc.tensor.matmul(
                                pzB[:],
                                lhsT=hT_prev[:, j * sh + t * P: j * sh + (t + 1) * P],
                                rhs=omaI[:, j * D:(j + 1) * D],
                                start=False, stop=(j == DJ - 1))
                    if layer < 0:
                        idx_tt = idx_sb[:, t * S * 8:(t + 1) * S * 8]
                        nc.gpsimd.dma_gather(
                            out_ap=kgt[:], in_ap=k_full[:], idxs_ap=idx_tt,
                            num_idxs=ET, num_idxs_reg=ET, elem_size=D,
                            transpose=True, single_packet=False)
                    src_tab = v_full if layer < 0 else h_full[layer]
                    for (ca, cb) in (splits0 if t == 0 else splits):
                        nn_i = (cb - ca) * P
                        idx_t = idx_sb[:, t * S * 8 + ca * 8: t * S * 8 + cb * 8]
                        nc.gpsimd.dma_gather(
                            out_ap=vg[:, ca:cb, :], in_ap=src_tab[:], idxs_ap=idx_t,
                            num_idxs=nn_i, num_idxs_reg=nn_i, elem_size=D,
                            single_packet=False)

                    pagg = ps.tile([P, D + 1], F32, name="pagg", tag="pagg", bufs=kpagg)
                    if layer >= 0:
                        # two transposed-scatter accumulators must sit in
                        # DIFFERENT psum banks (per-bank accumulation state)
                        pagg2 = ps.tile([P, P], F32, name="pagg2", tag="psc", bufs=kpsc)
                        sage_halves = [pagg[:, :P], pagg2[:]]
                    else:
                        sage_halves = None
                    kexp = int(os.environ.get("KEXP", "4"))  # chunks per exp call
                    # walrus rejects TensorScalarPtr on Pool; keep builds on DVE
                    kpool = int(os.environ.get("KPOOLB", "0"))
                    if layer < 0:
                        # batched scores: groups of kexp chunks share one psum
                        # bank and one exp() call (amortizes Act access lat.)
                        for c0 in range(0, S, kexp):
                            cn = min(kexp, S - c0)
                            pscw = ps.tile([P, kexp * P], F32, name="pscw",
                                           tag="psc", bufs=kpsc)
                            for ci in range(cn):
                                c = c0 + ci
                                for j in range(DJ):
                                    nc.tensor.matmul(
                                        pscw[:, ci * P:(ci + 1) * P],
                                        lhsT=kgt[:, j, c * P:(c + 1) * P],
                                        rhs=qT[j][:, t * P:(t + 1) * P],
                                        start=(j == 0), stop=(j == DJ - 1))
                            exps = smp.tile([P, kexp * P], BF16, name="exps")
                            nc.scalar.activation(exps[:, :cn * P], pscw[:, :cn * P],
                                                 mybir.ActivationFunctionType.Exp,
                                                 scale=scale)
                            for ci in range(cn):
                                c = c0 + ci
                                dcol = dstc[:, t * S + c: t * S + c + 1]
                                w_b = smp.tile([P, P], V_DT, name="w_b", tag="w_b")
                                if os.environ.get("KWBSTT", "1") == "1":
                                    nc.vector.scalar_tensor_tensor(
                                        out=w_b[:], in0=iota_b[:], scalar=dcol,
                                        in1=exps[:, ci * P:(ci + 1) * P],
                                        op0=mybir.AluOpType.is_equal,
                                        op1=mybir.AluOpType.mult)
                                else:
                                    # two simple all-bf16 ops hit the DVE 2x
                                    # path; the fused STT variant does not
                                    ind_t = smp.tile([P, P], BF16, name="ind_t",
                                                     tag="ind_t")
                                    nc.vector.tensor_scalar(
                                        out=ind_t[:], in0=iota_b[:], scalar1=dcol,
                                        scalar2=None, op0=mybir.AluOpType.is_equal)
                                    nc.vector.tensor_tensor(
                                        out=w_b[:], in0=ind_t[:],
                                        in1=exps[:, ci * P:(ci + 1) * P],
                                        op=mybir.AluOpType.mult)
                                nc.tensor.matmul(pagg[:, :D], lhsT=w_b[:],
                                                 rhs=vg[:, c, :],
                                                 start=(c == 0), stop=(c == S - 1))
                                nc.tensor.matmul(pagg[:, D:D + 1], lhsT=w_b[:],
                                                 rhs=ones_v[:],
                                                 start=False, stop=(c == S - 1))
                    if layer >= 0:
                        for c in range(S):
                            dcol = dstc[:, t * S + c: t * S + c + 1]
                            # scatter TRANSPOSED: paggT_j[d_j, dst]
                            # accumulates in two psum halves (distinct banks).
                            ind_b = smp.tile([P, P], H_DT, name="ind_b", tag="w_b")
                            nc.vector.tensor_scalar(
                                out=ind_b[:], in0=iota_b[:], scalar1=dcol,
                                scalar2=None, op0=mybir.AluOpType.is_equal)
                            for j in range(DJ):
                                nc.tensor.matmul(
                                    sage_halves[j],
                                    lhsT=vg[:, c, j * P:(j + 1) * P],
                                    rhs=ind_b[:],
                                    start=(c == 0), stop=(c == S - 1))

                    # ---- tile epilogue -> h_out tile [node, d] ----
                    if layer < 0:
                        smax = smp.tile([P, 1], F32, name="smax")
                        nc.vector.tensor_scalar(
                            out=smax[:], in0=pagg[:, D:D + 1], scalar1=1e-30,
                            scalar2=None, op0=mybir.AluOpType.max)
                        rs = smp.tile([P, 1], F32, name="rs")
                        nc.vector.reciprocal(rs[:], smax[:])
                        pskip = ps.tile([P, D], F32, name="pskip", tag="pmm", bufs=kpmm)
                        for ji in range(DJ):
                            nc.tensor.matmul(pskip[:], lhsT=xtile(ji, t),
                                             rhs=wslice(WS, ji),
                                             start=(ji == 0), stop=False)
                        nc.tensor.matmul(pskip[:], lhsT=ones1[:],
                                         rhs=brow[0:1, :D],
                                         start=False, stop=True)
                        t1 = smp.tile([P, D], F32, name="t1", tag="t1")
                        nc.scalar.activation(t1[:], pagg[:, :D],
                                             mybir.ActivationFunctionType.Copy,
                                             scale=rs[:, :1])
                        t2 = smp.tile([P, D], F32, name="t2", tag="t2")
                        nc.vector.tensor_tensor(out=t2[:], in0=t1[:], in1=pskip[:],
                                                op=mybir.AluOpType.add)
                        nc.scalar.activation(h_out[:, t * D:(t + 1) * D], t2[:],
                                             mybir.ActivationFunctionType.Relu)
                    else:
                        # mean-term (gamma folded into Wl'): pzA = sumT^T @ Wl'
                        pzA = ps.tile([P, D], F32, name="pzA", tag="pmm", bufs=kpmm)
                        for j in range(DJ):
                            mT = smp.tile([P, P], D_DT, name="mT", tag="mT")
                            nc.scalar.copy(out=mT[:], in_=sage_halves[j])
                            nc.tensor.matmul(pzA[:], lhsT=mT[:],
                                             rhs=wslice(WL[layer], j),
                                             start=(j == 0), stop=(j == DJ - 1))
                        # gate(z) = al*z_affine + (1-al)h_prev: pzB already
                        # holds root+bias+residual; add mean term / deg.
                        t1 = smp.tile([P, D], F32, name="t1s", tag="t2")
                        nc.vector.tensor_scalar(
                            out=t1[:], in0=pzA[:], scalar1=invd[:, t:t + 1],
                            scalar2=None, op0=mybir.AluOpType.mult)
                        t2 = smp.tile([P, D], F32, name="t2s", tag="t3")
                        nc.vector.tensor_tensor(out=t2[:], in0=t1[:], in1=pzB[:],
                                                op=mybir.AluOpType.add)
                        if layer == L - 1:
                            houtf = smp.tile([P, D], F32, name="houtf", tag="t4")
                            nc.scalar.activation(houtf[:], t2[:],
                                                 mybir.ActivationFunctionType.Relu)
                            nc.sync.dma_start(out=out_dram[t * P:(t + 1) * P, :],
                                              in_=houtf[:])
                        else:
                            nc.scalar.activation(h_out[:, t * D:(t + 1) * D], t2[:],
                                                 mybir.ActivationFunctionType.Relu)

                    if layer < L - 1:
                        nc.sync.dma_start(out=hag_in[li][t * P:(t + 1) * P, :],
                                          in_=h_out[:, t * D:(t + 1) * D])
                        for j in range(DJ):
                            ptr2 = ps.tile([P, P], BF16, name="ptr2", tag="ptr", bufs=kptr)
                            nc.tensor.transpose(
                                out=ptr2[:],
                                in_=h_out[:, t * D + j * P: t * D + (j + 1) * P],
                                identity=ident_b[:])
                            nc.scalar.copy(
                                out=hT_out[:, j * sh + t * P: j * sh + (t + 1) * P],
                                in_=ptr2[:])

                if layer < L - 1:
                    allgather(hag_in[li], h_full[li])

            if stages <= 1:
                # dump k_full slice so the program has an output
                tmpo = smp.tile([P, D], F32, name="tmpo")
                for t in range(nt):
                    nc.vector.tensor_copy(out=tmpo[:], in_=xt[:, :D])
                    nc.sync.dma_start(out=out_dram[t * P:(t + 1) * P, :], in_=tmpo[:])
            else:
                agg_pass(-1, None, None, h_cur, hT_cur)
                bufs = [(h_cur, hT_cur), (h_nxt, hT_nxt)]
                for i in range(min(L, stages - 2)):
                    h_prev, hT_prev = bufs[i % 2]
                    h_out, hT_out = bufs[(i + 1) % 2]
                    agg_pass(i, h_prev, hT_prev, h_out, hT_out)
                if stages - 2 < L:
                    hsrc, _ = bufs[max(0, stages - 2) % 2]
                    for t in range(nt):
                        tmpo = smp.tile([P, D], F32, name="tmpo2")
                        nc.vector.tensor_copy(out=tmpo[:],
                                              in_=hsrc[:, t * D:(t + 1) * D])
                        nc.sync.dma_start(out=out_dram[t * P:(t + 1) * P, :],
                                          in_=tmpo[:])

    nc.compile()
    _nc_cache[key] = nc
    return nc


def _balance_perm(dst, n, n_pad):
    """Renumber nodes so each dst-tile of 128 has a near-equal edge count.
    Returns perm: old id -> new id (padding slots filled with virtual ids)."""
    import heapq

    deg = np.bincount(dst, minlength=n)
    nbins = n_pad // P
    counts = np.zeros(nbins, np.int64)
    perm = np.empty(n, np.int64)
    heap = [(0, g) for g in range(nbins)]
    heapq.heapify(heap)
    for node in np.argsort(-deg, kind="stable"):
        while True:
            load, g = heapq.heappop(heap)
            if counts[g] < P:
                break
        perm[node] = g * P + counts[g]
        counts[g] += 1
        heapq.heappush(heap, (load + int(deg[node]), g))
    return perm


def _host_prep(x, src, dst, Wq, bq, Wk, bk, Wv, bv, Ws, bs, Wl, bl, Wr,
               gamma, beta, alpha_res):
    n, d = x.shape
    n_pad = ((n + NC * P - 1) // (NC * P)) * (NC * P)
    sh = n_pad // NC
    nt = sh // P
    n_tiles = n_pad // P

    perm = _balance_perm(dst, n, n_pad)
    src = perm[src]
    dst = perm[dst]

    order = np.argsort(dst, kind="stable")
    src_s, dst_s = src[order], dst[order]
    tile_of = dst_s // P
    counts = np.bincount(tile_of, minlength=n_tiles)
    starts = np.concatenate([[0], np.cumsum(counts)])
    S = int(max(1, (counts.max() + P - 1) // P))
    ET = S * P

    deg = np.bincount(dst, minlength=n_pad).astype(np.float32)
    invdeg_full = 1.0 / np.maximum(deg, 1.0)

    al = 1.0 / (1.0 + np.exp(-alpha_res))
    oma = float(1.0 - al)
    bn_scale = 1.0 / np.sqrt(1.0 + BN_EPS)
    scale = 1.0 / np.sqrt(float(d))

    x_pad = np.zeros((n_pad, D), np.float32)
    x_pad[perm] = x
    xT = x_pad.T.copy()

    Gx = [al * bn_scale * gamma[i] for i in range(L)]
    Bx = [al * (bl[i] * bn_scale * gamma[i] + beta[i]) for i in range(L)]

    # gamma/bn/gate scale folded into the SAGE weights (per-output-column)
    weights = [Wq, Wk, Wv, Ws,
               Wl[0] * Gx[0][None, :], Wr[0] * Gx[0][None, :],
               Wl[1] * Gx[1][None, :], Wr[1] * Gx[1][None, :],
               Wl[2] * Gx[2][None, :], Wr[2] * Gx[2][None, :]]
    wpack = np.zeros((P, 11 * DJ * D), D_NP)
    for w, W in enumerate(weights):
        for j in range(DJ):
            wpack[:, (w * DJ + j) * D:(w * DJ + j + 1) * D] = W[j * P:(j + 1) * P, :]
    for j in range(DJ):
        blk = np.zeros((P, D), np.float32)
        blk[np.arange(P), j * P + np.arange(P)] = oma
        wpack[:, (10 * DJ + j) * D:(10 * DJ + j + 1) * D] = blk
    vecs = [bk, bv, bs, Gx[0], Bx[0], Gx[1], Bx[1], Gx[2], Bx[2]]
    vpack = np.empty((P, 9 * D + DJ), np.float32)
    for k, v in enumerate(vecs):
        vpack[:, k * D:(k + 1) * D] = np.tile(v[None, :], (P, 1))
    for j in range(DJ):
        vpack[:, 9 * D + j] = bq[j * P:(j + 1) * P]

    in_maps = []
    for r in range(NC):
        idx_arr = np.zeros((P, nt * S * 8), np.int16)
        dst_arr = np.full((P, nt * S), 128.0, np.float32)
        for tloc in range(nt):
            g = r * nt + tloc
            e0, e1 = starts[g], starts[g + 1]
            cnt = e1 - e0
            srcs = np.zeros(ET, np.int64)
            srcs[:cnt] = src_s[e0:e1]
            dl = np.full(ET, 128, np.int64)
            dl[:cnt] = dst_s[e0:e1] - g * P
            idx_arr[:, tloc * S * 8:(tloc + 1) * S * 8] = _wrap_idx(srcs)
            dst_arr[:, tloc * S:(tloc + 1) * S] = dl.reshape(S, P).T
        invdeg_r = invdeg_full[r * sh:(r + 1) * sh].reshape(nt, P).T.copy()

        xt_r = np.empty((P, DJ * sh), D_NP)
        for j in range(DJ):
            xt_r[:, j * sh:(j + 1) * sh] = xT[j * P:(j + 1) * P, r * sh:(r + 1) * sh]

        in_maps.append({
            "xt_in": xt_r,
            "wpack_in": wpack,
            "vpack_in": vpack,
            "idx_in": idx_arr,
            "dst_in": dst_arr,
            "invdeg_in": np.ascontiguousarray(invdeg_r),
        })
    return in_maps, (n_pad, sh, nt, S, scale, oma), perm


def kernel(**inputs):
    x = np.asarray(inputs["x"], np.float32)
    edge_index = np.asarray(inputs["edge_index"])
    args = dict(
        Wq=np.asarray(inputs["Wq"], np.float32), bq=np.asarray(inputs["bq"], np.float32),
        Wk=np.asarray(inputs["Wk"], np.float32), bk=np.asarray(inputs["bk"], np.float32),
        Wv=np.asarray(inputs["Wv"], np.float32), bv=np.asarray(inputs["bv"], np.float32),
        Ws=np.asarray(inputs["Ws"], np.float32), bs=np.asarray(inputs["bs"], np.float32),
        Wl=np.asarray(inputs["Wl"], np.float32), bl=np.asarray(inputs["bl"], np.float32),
        Wr=np.asarray(inputs["Wr"], np.float32),
        gamma=np.asarray(inputs["gamma"], np.float32),
        beta=np.asarray(inputs["beta"], np.float32),
        alpha_res=float(np.asarray(inputs["alpha_res"])),
    )
    src = edge_index[0].astype(np.int64)
    dst = edge_index[1].astype(np.int64)

    in_maps, (n_pad, sh, nt, S, scale, oma), perm = _host_prep(x, src, dst, **args)
    t0 = time.time()
    nc = build_nc(n_pad, sh, nt, S, scale, oma)
    print(f"[kernel] build+compile {time.time()-t0:.1f}s", flush=True)
    t0 = time.time()
    res = run_bass_kernel_spmd(nc, in_maps, core_ids=list(range(NC)))
    print(f"[kernel] run {time.time()-t0:.1f}s", flush=True)
    out = np.concatenate([res.results[r]["out"] for r in range(NC)], axis=0)
    return out[perm]



# revision 83
# speedup vs baseline: 1.0108x; 1.0108x over previous
"""Trainium2 Bass kernel for nn_MixGNN (TransformerConv + 3x SAGEConv + BN + gated residual).

Strategy (8 NeuronCores, dst-node sharding):
  - Pad N 10000 -> 10240; core r owns 1280 dst nodes = 10 tiles of 128.
  - Host prep: degree-balanced node renumbering (each dst-tile gets ~E/80
    edges, shrinking the padded chunk count S), edges sorted by dst and
    bucketed per tile, wrapped int16 gather indices, per-chunk local-dst
    columns, 1/deg, bf16-packed weights with BN gamma/gate folded into
    Wl/Wr columns and an extra oma*I block for the PE-side residual.
  - Device per layer: dense matmuls on PE with bf16 operands (1 cycle/row);
    per-edge work via dma_gather of source rows (512B descriptors) +
    indicator matmuls (Ind[e,n] = (dst_e==n) built by DVE is_equal).
    Transformer: scores KgT.T @ qT from a transposed bf16 gather, batched
    exp() over 4 score chunks per Act call, w_b = Ind*exp via one fused DVE
    op, agg + exp-sum accumulated in one PSUM group; epilogue adds x@Ws with
    the bias via a rank-1 ones-row matmul. SAGE: TRANSPOSED scatter
    (paggT_j[d_j,dst] in two distinct PSUM banks), pzA = sumT^T @ Wl' with
    1/deg applied after (commutes), and pzB = h@Wr' + bias + oma*h computed
    entirely on PE (bias via ones-row, residual via oma*I against hT) and
    issued before the gathers so PE fills the wait; epilogue is 2 DVE ops +
    relu, with non-final h kept bf16 so relu output feeds the AllGather
    staging and the hT transposes directly.
  - Halo exchange: AllGather of each core's h shard (bf16) into a full table
    in shared DRAM before every aggregation.
Output: fp32 [10000, 256].
"""
import os
import sys
import time

import numpy as np

for _p in ("/opt/trn_rl_repo",):
    if _p not in sys.path:
        sys.path.insert(0, _p)

import ml_dtypes  # noqa: E402
import concourse.bacc as bacc  # noqa: E402
import concourse.mybir as mybir  # noqa: E402
import concourse.tile as tile  # noqa: E402
from concourse.bass_utils import run_bass_kernel_spmd  # noqa: E402

P = 128
D = 256
DJ = D // P           # 2 d-chunks of 128
NC = 8                # cores
L = 3                 # SAGE layers
BN_EPS = 1e-5

# dtype knobs for gathered tables (accuracy vs bandwidth)
V_BF16 = True         # v table + attention agg in bf16
H_BF16 = True         # h tables + SAGE agg in bf16

F32 = mybir.dt.float32
F32R = (mybir.dt.float32 if os.environ.get("KNOF32R") == "1"
        else mybir.dt.float32r)  # f32 bits, PE full-rate mode at >=256 free
BF16 = mybir.dt.bfloat16
I16 = mybir.dt.int16
# dense-path operand dtype: bf16 halves DMA + SBUF for x/W/hT/mT tables
DENSE_BF16 = os.environ.get("KDENSE", "bf16") == "bf16"
D_DT = BF16 if DENSE_BF16 else F32R
D_NP = ml_dtypes.bfloat16 if DENSE_BF16 else np.float32
V_DT = BF16 if V_BF16 else F32
H_DT = BF16 if H_BF16 else F32

_nc_cache = {}


def _wrap_idx(a):
    """[S*128] int array -> [128, S*8] int16 wrapped gather-index layout."""
    w16 = a.reshape(-1, 16).T.astype(np.int16)   # [16, S*8]
    return np.tile(w16, (8, 1))                  # replicate to 8 Q7 stripes


def build_nc(n_pad, sh, nt, S, scale, oma):
    stages = int(os.environ.get("KSTAGES", "5"))
    nocc = os.environ.get("KNOCC") == "1"
    ksm = int(os.environ.get("KSM", "10"))
    kgp = int(os.environ.get("KGP", "2"))
    kpsc = int(os.environ.get("KPSC", "3"))
    kptr = int(os.environ.get("KPTR", "1"))
    kpagg = int(os.environ.get("KPAGG", "2"))
    kpmm = int(os.environ.get("KPMM", "2"))
    khalf = int(os.environ.get("KHALF", "4"))  # gather splits per tile
    kabl = os.environ.get("KABL", "")
    key = (n_pad, sh, nt, S, round(scale, 9), round(oma, 9), V_BF16, H_BF16, stages,
           nocc, ksm, kgp, kpsc, kptr, kpagg, kpmm, khalf, kabl,
           os.environ.get("KHALFT"),
           os.environ.get("KKGT"), os.environ.get("KVG"),
           os.environ.get("KEXP"), os.environ.get("KPOOLB"),
           os.environ.get("KWBSTT"), os.environ.get("KGRAD"))
    if key in _nc_cache:
        return _nc_cache[key]

    ET = S * P  # padded edges per tile
    ndev = 1 if nocc else NC
    nc = bacc.Bacc("TRN2", target_bir_lowering=False, debug=False, num_devices=ndev)

    xt_in = nc.dram_tensor("xt_in", [P, DJ * sh], D_DT, kind="ExternalInput")
    wpack_in = nc.dram_tensor("wpack_in", [P, 11 * DJ * D], D_DT, kind="ExternalInput")
    vpack_in = nc.dram_tensor("vpack_in", [P, 9 * D + DJ], F32, kind="ExternalInput")
    idx_in = nc.dram_tensor("idx_in", [P, nt * S * 8], I16, kind="ExternalInput")
    dst_in = nc.dram_tensor("dst_in", [P, nt * S], F32, kind="ExternalInput")
    invdeg_in = nc.dram_tensor("invdeg_in", [P, nt], F32, kind="ExternalInput")
    out_dram = nc.dram_tensor("out", [sh, D], F32, kind="ExternalOutput")

    WQ, WK, WV, WS = 0, 1, 2, 3
    WL = [4, 6, 8]
    WR = [5, 7, 9]
    VBK, VBV, VBS = 0, 1, 2

    with tile.TileContext(nc) as tc:
        with (
            tc.tile_pool(name="cst", bufs=1) as cst,
            tc.tile_pool(name="sb", bufs=1) as sb,
            tc.tile_pool(name="g", bufs=kgp) as gp,
            tc.tile_pool(name="sm", bufs=ksm) as smp,
            tc.tile_pool(name="ps", bufs=2, space="PSUM") as ps,
            tc.tile_pool(name="dr", bufs=1, space="DRAM") as dr,
        ):
            # ---------------- constants / inputs to SBUF ----------------
            wp = cst.tile([P, 11 * DJ * D], D_DT)
            nc.sync.dma_start(out=wp[:], in_=wpack_in[:])
            vp = cst.tile([P, 9 * D + DJ], F32)
            nc.sync.dma_start(out=vp[:], in_=vpack_in[:])
            xt = cst.tile([P, DJ * sh], D_DT)
            for _xi in range(4):
                _c0 = _xi * (DJ * sh // 4)
                _c1 = (_xi + 1) * (DJ * sh // 4)
                nc.sync.dma_start(out=xt[:, _c0:_c1], in_=xt_in[:, _c0:_c1])
            dstc = cst.tile([P, nt * S], F32)
            nc.sync.dma_start(out=dstc[:], in_=dst_in[:])
            invd = cst.tile([P, nt], F32)
            nc.sync.dma_start(out=invd[:], in_=invdeg_in[:])
            idx_sb = cst.tile([P, nt * S * 8], I16)
            nc.sync.dma_start(out=idx_sb[:], in_=idx_in[:])

            iota_i = cst.tile([P, P], mybir.dt.int32)
            nc.gpsimd.iota(iota_i[:], pattern=[[1, P]], base=0, channel_multiplier=0)
            ones_v = cst.tile([P, 1], V_DT)
            nc.vector.memset(ones_v[:], 1.0)
            # identity for PE transposes: (iota_row == partition_idx)
            iota_part = cst.tile([P, 1], mybir.dt.int32)
            nc.gpsimd.iota(iota_part[:], pattern=[[1, 1]], base=0, channel_multiplier=1)
            iota_part_f = cst.tile([P, 1], F32)
            nc.vector.tensor_copy(out=iota_part_f[:], in_=iota_part[:])
            iota_b = cst.tile([P, P], BF16)
            nc.vector.tensor_copy(out=iota_b[:], in_=iota_i[:])
            ident_b = cst.tile([P, P], BF16)
            nc.vector.tensor_scalar(
                out=ident_b[:], in0=iota_b[:], scalar1=iota_part_f[:, :1],
                scalar2=None, op0=mybir.AluOpType.is_equal,
            )
            # rank-1 bias rows + oma*I blocks (bias/residual folded into PE)
            ones1 = cst.tile([1, P], BF16)
            nc.vector.memset(ones1[:], 1.0)
            brow = cst.tile([1, (L + 1) * D], BF16)
            nc.vector.tensor_copy(out=brow[0:1, :D], in_=vp[0:1, 2 * D:3 * D])
            for _i in range(L):
                nc.vector.tensor_copy(
                    out=brow[0:1, (_i + 1) * D:(_i + 2) * D],
                    in_=vp[0:1, (4 + 2 * _i) * D:(5 + 2 * _i) * D])
            omaI = wp[:, 10 * DJ * D:11 * DJ * D]  # host-packed oma*I blocks

            def wslice(w, j):
                return wp[:, (w * DJ + j) * D:(w * DJ + j + 1) * D]

            def vslice(k):
                return vp[:, k * D:(k + 1) * D]

            def xtile(j, t):
                return xt[:, j * sh + t * P: j * sh + (t + 1) * P]

            # ---------------- DRAM tables ----------------
            k_ag_in = dr.tile([sh, D], BF16)
            v_ag_in = dr.tile([sh, D], V_DT)
            k_full = dr.tile([n_pad, D], BF16, addr_space="Shared")
            v_full = dr.tile([n_pad, D], V_DT, addr_space="Shared")
            hag_in = [dr.tile([sh, D], H_DT, name=f"hag_in_{i}") for i in range(L)]
            h_full = [dr.tile([n_pad, D], H_DT, name=f"h_full_{i}", addr_space="Shared")
                      for i in range(L)]

            def allgather(in_t, out_t):
                if nocc:
                    nc.sync.dma_start(out=out_t[:sh], in_=in_t[:])
                else:
                    nc.gpsimd.collective_compute(
                        "AllGather", mybir.AluOpType.bypass,
                        replica_groups=[list(range(NC))],
                        ins=[in_t[:]], outs=[out_t[:]],
                    )

            # ---------------- stage 0: k,v shard tables + AG, then qT ----------------
            for t in range(nt):
                pk = ps.tile([P, D], F32, name="pk", tag="pmm", bufs=kpmm)
                for ji in range(DJ):
                    nc.tensor.matmul(pk[:], lhsT=xtile(ji, t), rhs=wslice(WK, ji),
                                     start=(ji == 0), stop=(ji == DJ - 1))
                k_sb = smp.tile([P, D], BF16, name="k_sb")
                nc.vector.tensor_tensor(out=k_sb[:], in0=pk[:], in1=vslice(VBK),
                                        op=mybir.AluOpType.add)
                nc.sync.dma_start(out=k_ag_in[t * P:(t + 1) * P, :], in_=k_sb[:])

                pv = ps.tile([P, D], F32, name="pv", tag="pmm", bufs=kpmm)
                for ji in range(DJ):
                    nc.tensor.matmul(pv[:], lhsT=xtile(ji, t), rhs=wslice(WV, ji),
                                     start=(ji == 0), stop=(ji == DJ - 1))
                v_sb = smp.tile([P, D], V_DT, name="v_sb")
                nc.vector.tensor_tensor(out=v_sb[:], in0=pv[:], in1=vslice(VBV),
                                        op=mybir.AluOpType.add)
                nc.sync.dma_start(out=v_ag_in[t * P:(t + 1) * P, :], in_=v_sb[:])

            allgather(k_ag_in, k_full)
            allgather(v_ag_in, v_full)

            qT = []
            for j in range(DJ):
                qTj = sb.tile([P, sh], BF16, name=f"qT_{j}")
                n0 = 0
                while n0 < sh:
                    nn = min(512, sh - n0)
                    pq = ps.tile([P, 512], F32, name="pq", tag="pmm", bufs=kpmm)
                    for ji in range(DJ):
                        nc.tensor.matmul(
                            pq[:, :nn],
                            lhsT=wslice(WQ, ji)[:, j * P:(j + 1) * P],
                            rhs=xt[:, ji * sh + n0: ji * sh + n0 + nn],
                            start=(ji == 0), stop=(ji == DJ - 1),
                        )
                    nc.vector.tensor_scalar(
                        out=qTj[:, n0:n0 + nn], in0=pq[:, :nn],
                        scalar1=vp[:, 9 * D + j: 9 * D + j + 1], scalar2=None,
                        op0=mybir.AluOpType.add,
                    )
                    n0 += nn
                qT.append(qTj)

            # shard-resident activations
            h_cur = sb.tile([P, nt * D], H_DT)
            h_nxt = sb.tile([P, nt * D], H_DT)
            hT_cur = sb.tile([P, DJ * sh], D_DT)
            hT_nxt = sb.tile([P, DJ * sh], D_DT)

            def agg_pass(layer, h_prev, hT_prev, h_out, hT_out):
                """layer -1: transformer (h_prev/hT_prev unused); 0..L-1: SAGE."""
                li = layer + 1  # h table index this pass WRITES (0 for transformer)
                kh = khalf if layer >= 0 else int(os.environ.get("KHALFT", "3"))
                splits = []  # (c0, c1) chunk ranges per gather piece
                base = (S + kh - 1) // kh
                c0 = 0
                while c0 < S:
                    splits.append((c0, min(S, c0 + base)))
                    c0 += base
                # graded head for tile 0: a small leading split lets chunk-0
                # consumers start right after the pass-gating AllGather
                kgrad = int(os.environ.get("KGRAD", "0"))
                splits0 = splits
                if kgrad and layer >= 0:
                    splits0 = [(0, kgrad)]
                    c0 = kgrad
                    while c0 < S:
                        splits0.append((c0, min(S, c0 + base)))
                        c0 += base
                for t in range(nt):
                    if layer < 0:
                        kgt = gp.tile([P, DJ, ET], BF16, name="kgt", tag="kgt",
                                      bufs=int(os.environ.get("KKGT", "2")))
                        vg = gp.tile([P, S, D], V_DT, name="vg", tag="vg",
                                     bufs=int(os.environ.get("KVG", "3")))
                        pzB = None
                    else:
                        kgt = None
                        vg = gp.tile([P, S, D], H_DT, name="hg", tag="vg",
                                     bufs=int(os.environ.get("KVG", "3")))
                        # root + bias + gated residual, all on PE — independent
                        # of the gather; issue first so PE fills the wait.
                        pzB = ps.tile([P, D], F32, name="pzB", tag="pmm", bufs=kpmm)
                        for j in range(DJ):
                            nc.tensor.matmul(
                                pzB[:],
                                lhsT=hT_prev[:, j * sh + t * P: j * sh + (t + 1) * P],
                                rhs=wslice(WR[layer], j),
                                start=(j == 0), stop=False)
                        nc.tensor.matmul(
                            pzB[:], lhsT=ones1[:],
                            rhs=brow[0:1, (layer + 1) * D:(layer + 2) * D],
                            start=False, stop=False)
                        for j in range(DJ):
                            nc.tensor.matmul(
                                pzB[:],
                                lhsT=hT_prev[:, j * sh + t * P: j * sh + (t + 1) * P],
                                rhs=omaI[:, j * D:(j + 1) * D],
                                start=False, stop=(j == DJ - 1))
                    if layer < 0:
                        idx_tt = idx_sb[:, t * S * 8:(t + 1) * S * 8]
                        nc.gpsimd.dma_gather(
                            out_ap=kgt[:], in_ap=k_full[:], idxs_ap=idx_tt,
                            num_idxs=ET, num_idxs_reg=ET, elem_size=D,
                            transpose=True, single_packet=False)
                    src_tab = v_full if layer < 0 else h_full[layer]
                    for (ca, cb) in (splits0 if t == 0 else splits):
                        nn_i = (cb - ca) * P
                        idx_t = idx_sb[:, t * S * 8 + ca * 8: t * S * 8 + cb * 8]
                        nc.gpsimd.dma_gather(
                            out_ap=vg[:, ca:cb, :], in_ap=src_tab[:], idxs_ap=idx_t,
                            num_idxs=nn_i, num_idxs_reg=nn_i, elem_size=D,
                            single_packet=False)

                    pagg = ps.tile([P, D + 1], F32, name="pagg", tag="pagg", bufs=kpagg)
                    if layer >= 0:
                        # two transposed-scatter accumulators must sit in
                        # DIFFERENT psum banks (per-bank accumulation state)
                        pagg2 = ps.tile([P, P], F32, name="pagg2", tag="psc", bufs=kpsc)
                        sage_halves = [pagg[:, :P], pagg2[:]]
                    else:
                        sage_halves = None
                    kexp = int(os.environ.get("KEXP", "4"))  # chunks per exp call
                    # walrus rejects TensorScalarPtr on Pool; keep builds on DVE
                    kpool = int(os.environ.get("KPOOLB", "0"))
                    if layer < 0:
                        # batched scores: groups of kexp chunks share one psum
                        # bank and one exp() call (amortizes Act access lat.)
                        for c0 in range(0, S, kexp):
                            cn = min(kexp, S - c0)
                            pscw = ps.tile([P, kexp * P], F32, name="pscw",
                                           tag="psc", bufs=kpsc)
                            for ci in range(cn):
                                c = c0 + ci
                                for j in range(DJ):
                                    nc.tensor.matmul(
                                        pscw[:, ci * P:(ci + 1) * P],
                                        lhsT=kgt[:, j, c * P:(c + 1) * P],
                                        rhs=qT[j][:, t * P:(t + 1) * P],
                                        start=(j == 0), stop=(j == DJ - 1))
                            exps = smp.tile([P, kexp * P], BF16, name="exps")
                            nc.scalar.activation(exps[:, :cn * P], pscw[:, :cn * P],
                                                 mybir.ActivationFunctionType.Exp,
                                                 scale=scale)
                            for ci in range(cn):
                                c = c0 + ci
                                dcol = dstc[:, t * S + c: t * S + c + 1]
                                w_b = smp.tile([P, P], V_DT, name="w_b", tag="w_b")
                                if os.environ.get("KWBSTT", "1") == "1":
                                    nc.vector.scalar_tensor_tensor(
                                        out=w_b[:], in0=iota_b[:], scalar=dcol,
                                        in1=exps[:, ci * P:(ci + 1) * P],
                                        op0=mybir.AluOpType.is_equal,
                                        op1=mybir.AluOpType.mult)
                                else:
                                    # two simple all-bf16 ops hit the DVE 2x
                                    # path; the fused STT variant does not
                                    ind_t = smp.tile([P, P], BF16, name="ind_t",
                                                     tag="ind_t")
                                    nc.vector.tensor_scalar(
                                        out=ind_t[:], in0=iota_b[:], scalar1=dcol,
                                        scalar2=None, op0=mybir.AluOpType.is_equal)
                                    nc.vector.tensor_tensor(
                                        out=w_b[:], in0=ind_t[:],
                                        in1=exps[:, ci * P:(ci + 1) * P],
                                        op=mybir.AluOpType.mult)
                                nc.tensor.matmul(pagg[:, :D], lhsT=w_b[:],
                                                 rhs=vg[:, c, :],
                                                 start=(c == 0), stop=(c == S - 1))
                                nc.tensor.matmul(pagg[:, D:D + 1], lhsT=w_b[:],
                                                 rhs=ones_v[:],
                                                 start=False, stop=(c == S - 1))
                    if layer >= 0:
                        for c in range(S):
                            dcol = dstc[:, t * S + c: t * S + c + 1]
                            # scatter TRANSPOSED: paggT_j[d_j, dst]
                            # accumulates in two psum halves (distinct banks).
                            ind_b = smp.tile([P, P], H_DT, name="ind_b", tag="w_b")
                            nc.vector.tensor_scalar(
                                out=ind_b[:], in0=iota_b[:], scalar1=dcol,
                                scalar2=None, op0=mybir.AluOpType.is_equal)
                            for j in range(DJ):
                                nc.tensor.matmul(
                                    sage_halves[j],
                                    lhsT=vg[:, c, j * P:(j + 1) * P],
                                    rhs=ind_b[:],
                                    start=(c == 0), stop=(c == S - 1))

                    # ---- tile epilogue -> h_out tile [node, d] ----
                    if layer < 0:
                        smax = smp.tile([P, 1], F32, name="smax")
                        nc.vector.tensor_scalar(
                            out=smax[:], in0=pagg[:, D:D + 1], scalar1=1e-30,
                            scalar2=None, op0=mybir.AluOpType.max)
                        rs = smp.tile([P, 1], F32, name="rs")
                        nc.vector.reciprocal(rs[:], smax[:])
                        pskip = ps.tile([P, D], F32, name="pskip", tag="pmm", bufs=kpmm)
                        for ji in range(DJ):
                            nc.tensor.matmul(pskip[:], lhsT=xtile(ji, t),
                                             rhs=wslice(WS, ji),
                                             start=(ji == 0), stop=False)
                        nc.tensor.matmul(pskip[:], lhsT=ones1[:],
                                         rhs=brow[0:1, :D],
                                         start=False, stop=True)
                        t1 = smp.tile([P, D], F32, name="t1", tag="t1")
                        nc.scalar.activation(t1[:], pagg[:, :D],
                                             mybir.ActivationFunctionType.Copy,
                                             scale=rs[:, :1])
                        t2 = smp.tile([P, D], F32, name="t2", tag="t2")
                        nc.vector.tensor_tensor(out=t2[:], in0=t1[:], in1=pskip[:],
                                                op=mybir.AluOpType.add)
                        nc.scalar.activation(h_out[:, t * D:(t + 1) * D], t2[:],
                                             mybir.ActivationFunctionType.Relu)
                    else:
                        # mean-term (gamma folded into Wl'): pzA = sumT^T @ Wl'
                        pzA = ps.tile([P, D], F32, name="pzA", tag="pmm", bufs=kpmm)
                        for j in range(DJ):
                            mT = smp.tile([P, P], D_DT, name="mT", tag="mT")
                            nc.scalar.copy(out=mT[:], in_=sage_halves[j])
                            nc.tensor.matmul(pzA[:], lhsT=mT[:],
                                             rhs=wslice(WL[layer], j),
                                             start=(j == 0), stop=(j == DJ - 1))
                        # gate(z) = al*z_affine + (1-al)h_prev: pzB already
                        # holds root+bias+residual; add mean term / deg.
                        t1 = smp.tile([P, D], F32, name="t1s", tag="t2")
                        nc.vector.tensor_scalar(
                            out=t1[:], in0=pzA[:], scalar1=invd[:, t:t + 1],
                            scalar2=None, op0=mybir.AluOpType.mult)
                        t2 = smp.tile([P, D], F32, name="t2s", tag="t3")
                        nc.vector.tensor_tensor(out=t2[:], in0=t1[:], in1=pzB[:],
                                                op=mybir.AluOpType.add)
                        if layer == L - 1:
                            houtf = smp.tile([P, D], F32, name="houtf", tag="t4")
                            nc.scalar.activation(houtf[:], t2[:],
                                                 mybir.ActivationFunctionType.Relu)
                            nc.sync.dma_start(out=out_dram[t * P:(t + 1) * P, :],
                                              in_=houtf[:])
                        else:
                            nc.scalar.activation(h_out[:, t * D:(t + 1) * D], t2[:],
                                                 mybir.ActivationFunctionType.Relu)

                    if layer < L - 1:
                        nc.sync.dma_start(out=hag_in[li][t * P:(t + 1) * P, :],
                                          in_=h_out[:, t * D:(t + 1) * D])
                        for j in range(DJ):
                            ptr2 = ps.tile([P, P], BF16, name="ptr2", tag="ptr", bufs=kptr)
                            nc.tensor.transpose(
                                out=ptr2[:],
                                in_=h_out[:, t * D + j * P: t * D + (j + 1) * P],
                                identity=ident_b[:])
                            nc.scalar.copy(
                                out=hT_out[:, j * sh + t * P: j * sh + (t + 1) * P],
                                in_=ptr2[:])

                if layer < L - 1:
                    allgather(hag_in[li], h_full[li])

            if stages <= 1:
                # dump k_full slice so the program has an output
                tmpo = smp.tile([P, D], F32, name="tmpo")
                for t in range(nt):
                    nc.vector.tensor_copy(out=tmpo[:], in_=xt[:, :D])
                    nc.sync.dma_start(out=out_dram[t * P:(t + 1) * P, :], in_=tmpo[:])
            else:
                agg_pass(-1, None, None, h_cur, hT_cur)
                bufs = [(h_cur, hT_cur), (h_nxt, hT_nxt)]
                for i in range(min(L, stages - 2)):
                    h_prev, hT_prev = bufs[i % 2]
                    h_out, hT_out = bufs[(i + 1) % 2]
                    agg_pass(i, h_prev, hT_prev, h_out, hT_out)
                if stages - 2 < L:
                    hsrc, _ = bufs[max(0, stages - 2) % 2]
                    for t in range(nt):
                        tmpo = smp.tile([P, D], F32, name="tmpo2")
                        nc.vector.tensor_copy(out=tmpo[:],
                                              in_=hsrc[:, t * D:(t + 1) * D])
                        nc.sync.dma_start(out=out_dram[t * P:(t + 1) * P, :],
                                          in_=tmpo[:])

    nc.compile()
    _nc_cache[key] = nc
    return nc


def _balance_perm(dst, n, n_pad):
    """Renumber nodes so each dst-tile of 128 has a near-equal edge count.
    Returns perm: old id -> new id (padding slots filled with virtual ids)."""
    import heapq

    deg = np.bincount(dst, minlength=n)
    nbins = n_pad // P
    counts = np.zeros(nbins, np.int64)
    perm = np.empty(n, np.int64)
    heap = [(0, g) for g in range(nbins)]
    heapq.heapify(heap)
    for node in np.argsort(-deg, kind="stable"):
        while True:
            load, g = heapq.heappop(heap)
            if counts[g] < P:
                break
        perm[node] = g * P + counts[g]
        counts[g] += 1
        heapq.heappush(heap, (load + int(deg[node]), g))
    return perm


def _host_prep(x, src, dst, Wq, bq, Wk, bk, Wv, bv, Ws, bs, Wl, bl, Wr,
               gamma, beta, alpha_res):
    n, d = x.shape
    n_pad = ((n + NC * P - 1) // (NC * P)) * (NC * P)
    sh = n_pad // NC
    nt = sh // P
    n_tiles = n_pad // P

    perm = _balance_perm(dst, n, n_pad)
    src = perm[src]
    dst = perm[dst]

    order = np.argsort(dst, kind="stable")
    src_s, dst_s = src[order], dst[order]
    tile_of = dst_s // P
    counts = np.bincount(tile_of, minlength=n_tiles)
    starts = np.concatenate([[0], np.cumsum(counts)])
    S = int(max(1, (counts.max() + P - 1) // P))
    ET = S * P

    deg = np.bincount(dst, minlength=n_pad).astype(np.float32)
    invdeg_full = 1.0 / np.maximum(deg, 1.0)

    al = 1.0 / (1.0 + np.exp(-alpha_res))
    oma = float(1.0 - al)
    bn_scale = 1.0 / np.sqrt(1.0 + BN_EPS)
    scale = 1.0 / np.sqrt(float(d))

    x_pad = np.zeros((n_pad, D), np.float32)
    x_pad[perm] = x
    xT = x_pad.T.copy()

    Gx = [al * bn_scale * gamma[i] for i in range(L)]
    Bx = [al * (bl[i] * bn_scale * gamma[i] + beta[i]) for i in range(L)]

    # gamma/bn/gate scale folded into the SAGE weights (per-output-column)
    weights = [Wq, Wk, Wv, Ws,
               Wl[0] * Gx[0][None, :], Wr[0] * Gx[0][None, :],
               Wl[1] * Gx[1][None, :], Wr[1] * Gx[1][None, :],
               Wl[2] * Gx[2][None, :], Wr[2] * Gx[2][None, :]]
    wpack = np.zeros((P, 11 * DJ * D), D_NP)
    for w, W in enumerate(weights):
        for j in range(DJ):
            wpack[:, (w * DJ + j) * D:(w * DJ + j + 1) * D] = W[j * P:(j + 1) * P, :]
    for j in range(DJ):
        blk = np.zeros((P, D), np.float32)
        blk[np.arange(P), j * P + np.arange(P)] = oma
        wpack[:, (10 * DJ + j) * D:(10 * DJ + j + 1) * D] = blk
    vecs = [bk, bv, bs, Gx[0], Bx[0], Gx[1], Bx[1], Gx[2], Bx[2]]
    vpack = np.empty((P, 9 * D + DJ), np.float32)
    for k, v in enumerate(vecs):
        vpack[:, k * D:(k + 1) * D] = np.tile(v[None, :], (P, 1))
    for j in range(DJ):
        vpack[:, 9 * D + j] = bq[j * P:(j + 1) * P]

    in_maps = []
    for r in range(NC):
        idx_arr = np.zeros((P, nt * S * 8), np.int16)
        dst_arr = np.full((P, nt * S), 128.0, np.float32)
        for tloc in range(nt):
            g = r * nt + tloc
            e0, e1 = starts[g], starts[g + 1]
            cnt = e1 - e0
            srcs = np.zeros(ET, np.int64)
            srcs[:cnt] = src_s[e0:e1]
            dl = np.full(ET, 128, np.int64)
            dl[:cnt] = dst_s[e0:e1] - g * P
            idx_arr[:, tloc * S * 8:(tloc + 1) * S * 8] = _wrap_idx(srcs)
            dst_arr[:, tloc * S:(tloc + 1) * S] = dl.reshape(S, P).T
        invdeg_r = invdeg_full[r * sh:(r + 1) * sh].reshape(nt, P).T.copy()

        xt_r = np.empty((P, DJ * sh), D_NP)
        for j in range(DJ):
            xt_r[:, j * sh:(j + 1) * sh] = xT[j * P:(j + 1) * P, r * sh:(r + 1) * sh]

        in_maps.append({
            "xt_in": xt_r,
            "wpack_in": wpack,
            "vpack_in": vpack,
            "idx_in": idx_arr,
            "dst_in": dst_arr,
            "invdeg_in": np.ascontiguousarray(invdeg_r),
        })
    return in_maps, (n_pad, sh, nt, S, scale, oma), perm


def kernel(**inputs):
    x = np.asarray(inputs["x"], np.float32)
    edge_index = np.asarray(inputs["edge_index"])
    args = dict(
        Wq=np.asarray(inputs["Wq"], np.float32), bq=np.asarray(inputs["bq"], np.float32),
        Wk=np.asarray(inputs["Wk"], np.float32), bk=np.asarray(inputs["bk"], np.float32),
        Wv=np.asarray(inputs["Wv"], np.float32), bv=np.asarray(inputs["bv"], np.float32),
        Ws=np.asarray(inputs["Ws"], np.float32), bs=np.asarray(inputs["bs"], np.float32),
        Wl=np.asarray(inputs["Wl"], np.float32), bl=np.asarray(inputs["bl"], np.float32),
        Wr=np.asarray(inputs["Wr"], np.float32),
        gamma=np.asarray(inputs["gamma"], np.float32),
        beta=np.asarray(inputs["beta"], np.float32),
        alpha_res=float(np.asarray(inputs["alpha_res"])),
    )
    src = edge_index[0].astype(np.int64)
    dst = edge_index[1].astype(np.int64)

    in_maps, (n_pad, sh, nt, S, scale, oma), perm = _host_prep(x, src, dst, **args)
    t0 = time.time()
    nc = build_nc(n_pad, sh, nt, S, scale, oma)
    print(f"[kernel] build+compile {time.time()-t0:.1f}s", flush=True)
    t0 = time.time()
    res = run_bass_kernel_spmd(nc, in_maps, core_ids=list(range(NC)))
    print(f"[kernel] run {time.time()-t0:.1f}s", flush=True)
    out = np.concatenate([res.results[r]["out"] for r in range(NC)], axis=0)
    return out[perm]



# revision 89
# speedup vs baseline: 1.0120x; 1.0012x over previous
"""Trainium2 Bass kernel for nn_MixGNN (TransformerConv + 3x SAGEConv + BN + gated residual).

Strategy (8 NeuronCores, dst-node sharding):
  - Pad N 10000 -> 10240; core r owns 1280 dst nodes = 10 tiles of 128.
  - Host prep: degree-balanced node renumbering (each dst-tile gets ~E/80
    edges, shrinking the padded chunk count S), edges sorted by dst and
    bucketed per tile, wrapped int16 gather indices, per-chunk local-dst
    columns, 1/deg, bf16-packed weights with BN gamma/gate folded into
    Wl/Wr columns and an extra oma*I block for the PE-side residual.
  - Device per layer: dense matmuls on PE with bf16 operands (1 cycle/row);
    per-edge work via dma_gather of source rows (512B descriptors) +
    indicator matmuls (Ind[e,n] = (dst_e==n) built by DVE is_equal).
    Transformer: scores KgT.T @ qT from a transposed bf16 gather, batched
    exp() over 4 score chunks per Act call, w_b = Ind*exp via one fused DVE
    op, agg + exp-sum accumulated in one PSUM group; epilogue adds x@Ws with
    the bias via a rank-1 ones-row matmul. SAGE: TRANSPOSED scatter
    (paggT_j[d_j,dst] in two distinct PSUM banks), pzA = sumT^T @ Wl' with
    1/deg applied after (commutes), and pzB = h@Wr' + bias + oma*h computed
    entirely on PE (bias via ones-row, residual via oma*I against hT) and
    issued before the gathers so PE fills the wait; epilogue is 2 DVE ops +
    relu, with non-final h kept bf16 so relu output feeds the AllGather
    staging and the hT transposes directly.
  - Halo exchange: AllGather of each core's h shard (bf16) into a full table
    in shared DRAM before every aggregation.
Output: fp32 [10000, 256].
"""
import os
import sys
import time

import numpy as np

for _p in ("/opt/trn_rl_repo",):
    if _p not in sys.path:
        sys.path.insert(0, _p)

import ml_dtypes  # noqa: E402
import concourse.bacc as bacc  # noqa: E402
import concourse.mybir as mybir  # noqa: E402
import concourse.tile as tile  # noqa: E402
from concourse.bass_utils import run_bass_kernel_spmd  # noqa: E402

P = 128
D = 256
DJ = D // P           # 2 d-chunks of 128
NC = 8                # cores
L = 3                 # SAGE layers
BN_EPS = 1e-5

# dtype knobs for gathered tables (accuracy vs bandwidth)
V_BF16 = True         # v table + attention agg in bf16
H_BF16 = True         # h tables + SAGE agg in bf16

F32 = mybir.dt.float32
F32R = (mybir.dt.float32 if os.environ.get("KNOF32R") == "1"
        else mybir.dt.float32r)  # f32 bits, PE full-rate mode at >=256 free
BF16 = mybir.dt.bfloat16
I16 = mybir.dt.int16
# dense-path operand dtype: bf16 halves DMA + SBUF for x/W/hT/mT tables
DENSE_BF16 = os.environ.get("KDENSE", "bf16") == "bf16"
D_DT = BF16 if DENSE_BF16 else F32R
D_NP = ml_dtypes.bfloat16 if DENSE_BF16 else np.float32
V_DT = BF16 if V_BF16 else F32
H_DT = BF16 if H_BF16 else F32

_nc_cache = {}


def _wrap_idx(a):
    """[S*128] int array -> [128, S*8] int16 wrapped gather-index layout."""
    w16 = a.reshape(-1, 16).T.astype(np.int16)   # [16, S*8]
    return np.tile(w16, (8, 1))                  # replicate to 8 Q7 stripes


def build_nc(n_pad, sh, nt, S, scale, oma):
    stages = int(os.environ.get("KSTAGES", "5"))
    nocc = os.environ.get("KNOCC") == "1"
    ksm = int(os.environ.get("KSM", "10"))
    kgp = int(os.environ.get("KGP", "2"))
    kpsc = int(os.environ.get("KPSC", "3"))
    kptr = int(os.environ.get("KPTR", "2"))
    kpagg = int(os.environ.get("KPAGG", "1"))
    kpmm = int(os.environ.get("KPMM", "2"))
    khalf = int(os.environ.get("KHALF", "4"))  # gather splits per tile
    kabl = os.environ.get("KABL", "")
    key = (n_pad, sh, nt, S, round(scale, 9), round(oma, 9), V_BF16, H_BF16, stages,
           nocc, ksm, kgp, kpsc, kptr, kpagg, kpmm, khalf, kabl,
           os.environ.get("KHALFT"),
           os.environ.get("KKGT"), os.environ.get("KVG"),
           os.environ.get("KEXP"), os.environ.get("KPOOLB"),
           os.environ.get("KWBSTT"), os.environ.get("KGRAD"))
    if key in _nc_cache:
        return _nc_cache[key]

    ET = S * P  # padded edges per tile
    ndev = 1 if nocc else NC
    nc = bacc.Bacc("TRN2", target_bir_lowering=False, debug=False, num_devices=ndev)

    xt_in = nc.dram_tensor("xt_in", [P, DJ * sh], D_DT, kind="ExternalInput")
    wpack_in = nc.dram_tensor("wpack_in", [P, 11 * DJ * D], D_DT, kind="ExternalInput")
    vpack_in = nc.dram_tensor("vpack_in", [P, 9 * D + DJ], F32, kind="ExternalInput")
    idx_in = nc.dram_tensor("idx_in", [P, nt * S * 8], I16, kind="ExternalInput")
    dst_in = nc.dram_tensor("dst_in", [P, nt * S], F32, kind="ExternalInput")
    invdeg_in = nc.dram_tensor("invdeg_in", [P, nt], F32, kind="ExternalInput")
    out_dram = nc.dram_tensor("out", [sh, D], F32, kind="ExternalOutput")

    WQ, WK, WV, WS = 0, 1, 2, 3
    WL = [4, 6, 8]
    WR = [5, 7, 9]
    VBK, VBV, VBS = 0, 1, 2

    with tile.TileContext(nc) as tc:
        with (
            tc.tile_pool(name="cst", bufs=1) as cst,
            tc.tile_pool(name="sb", bufs=1) as sb,
            tc.tile_pool(name="g", bufs=kgp) as gp,
            tc.tile_pool(name="sm", bufs=ksm) as smp,
            tc.tile_pool(name="ps", bufs=2, space="PSUM") as ps,
            tc.tile_pool(name="dr", bufs=1, space="DRAM") as dr,
        ):
            # ---------------- constants / inputs to SBUF ----------------
            wp = cst.tile([P, 11 * DJ * D], D_DT)
            nc.sync.dma_start(out=wp[:], in_=wpack_in[:])
            vp = cst.tile([P, 9 * D + DJ], F32)
            nc.sync.dma_start(out=vp[:], in_=vpack_in[:])
            xt = cst.tile([P, DJ * sh], D_DT)
            for _xi in range(4):
                _c0 = _xi * (DJ * sh // 4)
                _c1 = (_xi + 1) * (DJ * sh // 4)
                nc.sync.dma_start(out=xt[:, _c0:_c1], in_=xt_in[:, _c0:_c1])
            dstc = cst.tile([P, nt * S], F32)
            nc.sync.dma_start(out=dstc[:], in_=dst_in[:])
            invd = cst.tile([P, nt], F32)
            nc.sync.dma_start(out=invd[:], in_=invdeg_in[:])
            idx_sb = cst.tile([P, nt * S * 8], I16)
            nc.sync.dma_start(out=idx_sb[:], in_=idx_in[:])

            iota_i = cst.tile([P, P], mybir.dt.int32)
            nc.gpsimd.iota(iota_i[:], pattern=[[1, P]], base=0, channel_multiplier=0)
            ones_v = cst.tile([P, 1], V_DT)
            nc.vector.memset(ones_v[:], 1.0)
            # identity for PE transposes: (iota_row == partition_idx)
            iota_part = cst.tile([P, 1], mybir.dt.int32)
            nc.gpsimd.iota(iota_part[:], pattern=[[1, 1]], base=0, channel_multiplier=1)
            iota_part_f = cst.tile([P, 1], F32)
            nc.vector.tensor_copy(out=iota_part_f[:], in_=iota_part[:])
            iota_b = cst.tile([P, P], BF16)
            nc.vector.tensor_copy(out=iota_b[:], in_=iota_i[:])
            ident_b = cst.tile([P, P], BF16)
            nc.vector.tensor_scalar(
                out=ident_b[:], in0=iota_b[:], scalar1=iota_part_f[:, :1],
                scalar2=None, op0=mybir.AluOpType.is_equal,
            )
            # rank-1 bias rows + oma*I blocks (bias/residual folded into PE)
            ones1 = cst.tile([1, P], BF16)
            nc.vector.memset(ones1[:], 1.0)
            brow = cst.tile([1, (L + 1) * D], BF16)
            nc.vector.tensor_copy(out=brow[0:1, :D], in_=vp[0:1, 2 * D:3 * D])
            for _i in range(L):
                nc.vector.tensor_copy(
                    out=brow[0:1, (_i + 1) * D:(_i + 2) * D],
                    in_=vp[0:1, (4 + 2 * _i) * D:(5 + 2 * _i) * D])
            omaI = wp[:, 10 * DJ * D:11 * DJ * D]  # host-packed oma*I blocks

            def wslice(w, j):
                return wp[:, (w * DJ + j) * D:(w * DJ + j + 1) * D]

            def vslice(k):
                return vp[:, k * D:(k + 1) * D]

            def xtile(j, t):
                return xt[:, j * sh + t * P: j * sh + (t + 1) * P]

            # ---------------- DRAM tables ----------------
            k_ag_in = dr.tile([sh, D], BF16)
            v_ag_in = dr.tile([sh, D], V_DT)
            k_full = dr.tile([n_pad, D], BF16, addr_space="Shared")
            v_full = dr.tile([n_pad, D], V_DT, addr_space="Shared")
            hag_in = [dr.tile([sh, D], H_DT, name=f"hag_in_{i}") for i in range(L)]
            h_full = [dr.tile([n_pad, D], H_DT, name=f"h_full_{i}", addr_space="Shared")
                      for i in range(L)]

            def allgather(in_t, out_t):
                if nocc:
                    nc.sync.dma_start(out=out_t[:sh], in_=in_t[:])
                else:
                    nc.gpsimd.collective_compute(
                        "AllGather", mybir.AluOpType.bypass,
                        replica_groups=[list(range(NC))],
                        ins=[in_t[:]], outs=[out_t[:]],
                    )

            # ---------------- stage 0: k,v shard tables + AG, then qT ----------------
            for t in range(nt):
                pk = ps.tile([P, D], F32, name="pk", tag="pmm", bufs=kpmm)
                for ji in range(DJ):
                    nc.tensor.matmul(pk[:], lhsT=xtile(ji, t), rhs=wslice(WK, ji),
                                     start=(ji == 0), stop=(ji == DJ - 1))
                k_sb = smp.tile([P, D], BF16, name="k_sb")
                nc.vector.tensor_tensor(out=k_sb[:], in0=pk[:], in1=vslice(VBK),
                                        op=mybir.AluOpType.add)
                nc.sync.dma_start(out=k_ag_in[t * P:(t + 1) * P, :], in_=k_sb[:])

                pv = ps.tile([P, D], F32, name="pv", tag="pmm", bufs=kpmm)
                for ji in range(DJ):
                    nc.tensor.matmul(pv[:], lhsT=xtile(ji, t), rhs=wslice(WV, ji),
                                     start=(ji == 0), stop=(ji == DJ - 1))
                v_sb = smp.tile([P, D], V_DT, name="v_sb")
                nc.vector.tensor_tensor(out=v_sb[:], in0=pv[:], in1=vslice(VBV),
                                        op=mybir.AluOpType.add)
                nc.sync.dma_start(out=v_ag_in[t * P:(t + 1) * P, :], in_=v_sb[:])

            allgather(k_ag_in, k_full)
            allgather(v_ag_in, v_full)

            qT = []
            for j in range(DJ):
                qTj = sb.tile([P, sh], BF16, name=f"qT_{j}")
                n0 = 0
                while n0 < sh:
                    nn = min(512, sh - n0)
                    pq = ps.tile([P, 512], F32, name="pq", tag="pmm", bufs=kpmm)
                    for ji in range(DJ):
                        nc.tensor.matmul(
                            pq[:, :nn],
                            lhsT=wslice(WQ, ji)[:, j * P:(j + 1) * P],
                            rhs=xt[:, ji * sh + n0: ji * sh + n0 + nn],
                            start=(ji == 0), stop=(ji == DJ - 1),
                        )
                    nc.vector.tensor_scalar(
                        out=qTj[:, n0:n0 + nn], in0=pq[:, :nn],
                        scalar1=vp[:, 9 * D + j: 9 * D + j + 1], scalar2=None,
                        op0=mybir.AluOpType.add,
                    )
                    n0 += nn
                qT.append(qTj)

            # shard-resident activations
            h_cur = sb.tile([P, nt * D], H_DT)
            h_nxt = sb.tile([P, nt * D], H_DT)
            hT_cur = sb.tile([P, DJ * sh], D_DT)
            hT_nxt = sb.tile([P, DJ * sh], D_DT)

            def agg_pass(layer, h_prev, hT_prev, h_out, hT_out):
                """layer -1: transformer (h_prev/hT_prev unused); 0..L-1: SAGE."""
                li = layer + 1  # h table index this pass WRITES (0 for transformer)
                kh = khalf if layer >= 0 else int(os.environ.get("KHALFT", "3"))
                splits = []  # (c0, c1) chunk ranges per gather piece
                base = (S + kh - 1) // kh
                c0 = 0
                while c0 < S:
                    splits.append((c0, min(S, c0 + base)))
                    c0 += base
                # graded head for tile 0: a small leading split lets chunk-0
                # consumers start right after the pass-gating AllGather
                kgrad = int(os.environ.get("KGRAD", "0"))
                splits0 = splits
                if kgrad and layer >= 0:
                    splits0 = [(0, kgrad)]
                    c0 = kgrad
                    while c0 < S:
                        splits0.append((c0, min(S, c0 + base)))
                        c0 += base
                for t in range(nt):
                    if layer < 0:
                        kgt = gp.tile([P, DJ, ET], BF16, name="kgt", tag="kgt",
                                      bufs=int(os.environ.get("KKGT", "2")))
                        vg = gp.tile([P, S, D], V_DT, name="vg", tag="vg",
                                     bufs=int(os.environ.get("KVG", "3")))
                        pzB = None
                    else:
                        kgt = None
                        vg = gp.tile([P, S, D], H_DT, name="hg", tag="vg",
                                     bufs=int(os.environ.get("KVG", "3")))
                        # root + bias + gated residual, all on PE — independent
                        # of the gather; issue first so PE fills the wait.
                        pzB = ps.tile([P, D], F32, name="pzB", tag="pmm", bufs=kpmm)
                        for j in range(DJ):
                            nc.tensor.matmul(
                                pzB[:],
                                lhsT=hT_prev[:, j * sh + t * P: j * sh + (t + 1) * P],
                                rhs=wslice(WR[layer], j),
                                start=(j == 0), stop=False)
                        nc.tensor.matmul(
                            pzB[:], lhsT=ones1[:],
                            rhs=brow[0:1, (layer + 1) * D:(layer + 2) * D],
                            start=False, stop=False)
                        for j in range(DJ):
                            nc.tensor.matmul(
                                pzB[:],
                                lhsT=hT_prev[:, j * sh + t * P: j * sh + (t + 1) * P],
                                rhs=omaI[:, j * D:(j + 1) * D],
                                start=False, stop=(j == DJ - 1))
                    if layer < 0:
                        idx_tt = idx_sb[:, t * S * 8:(t + 1) * S * 8]
                        nc.gpsimd.dma_gather(
                            out_ap=kgt[:], in_ap=k_full[:], idxs_ap=idx_tt,
                            num_idxs=ET, num_idxs_reg=ET, elem_size=D,
                            transpose=True, single_packet=False)
                    src_tab = v_full if layer < 0 else h_full[layer]
                    for (ca, cb) in (splits0 if t == 0 else splits):
                        nn_i = (cb - ca) * P
                        idx_t = idx_sb[:, t * S * 8 + ca * 8: t * S * 8 + cb * 8]
                        nc.gpsimd.dma_gather(
                            out_ap=vg[:, ca:cb, :], in_ap=src_tab[:], idxs_ap=idx_t,
                            num_idxs=nn_i, num_idxs_reg=nn_i, elem_size=D,
                            single_packet=False)

                    pagg = ps.tile([P, D + 1], F32, name="pagg", tag="pagg", bufs=kpagg)
                    if layer >= 0:
                        # two transposed-scatter accumulators must sit in
                        # DIFFERENT psum banks (per-bank accumulation state)
                        pagg2 = ps.tile([P, P], F32, name="pagg2", tag="psc", bufs=kpsc)
                        sage_halves = [pagg[:, :P], pagg2[:]]
                    else:
                        sage_halves = None
                    kexp = int(os.environ.get("KEXP", "4"))  # chunks per exp call
                    # walrus rejects TensorScalarPtr on Pool; keep builds on DVE
                    kpool = int(os.environ.get("KPOOLB", "0"))
                    if layer < 0:
                        # batched scores: groups of kexp chunks share one psum
                        # bank and one exp() call (amortizes Act access lat.)
                        for c0 in range(0, S, kexp):
                            cn = min(kexp, S - c0)
                            pscw = ps.tile([P, kexp * P], F32, name="pscw",
                                           tag="psc", bufs=kpsc)
                            for ci in range(cn):
                                c = c0 + ci
                                for j in range(DJ):
                                    nc.tensor.matmul(
                                        pscw[:, ci * P:(ci + 1) * P],
                                        lhsT=kgt[:, j, c * P:(c + 1) * P],
                                        rhs=qT[j][:, t * P:(t + 1) * P],
                                        start=(j == 0), stop=(j == DJ - 1))
                            exps = smp.tile([P, kexp * P], BF16, name="exps")
                            nc.scalar.activation(exps[:, :cn * P], pscw[:, :cn * P],
                                                 mybir.ActivationFunctionType.Exp,
                                                 scale=scale)
                            for ci in range(cn):
                                c = c0 + ci
                                dcol = dstc[:, t * S + c: t * S + c + 1]
                                w_b = smp.tile([P, P], V_DT, name="w_b", tag="w_b")
                                if os.environ.get("KWBSTT", "1") == "1":
                                    nc.vector.scalar_tensor_tensor(
                                        out=w_b[:], in0=iota_b[:], scalar=dcol,
                                        in1=exps[:, ci * P:(ci + 1) * P],
                                        op0=mybir.AluOpType.is_equal,
                                        op1=mybir.AluOpType.mult)
                                else:
                                    # two simple all-bf16 ops hit the DVE 2x
                                    # path; the fused STT variant does not
                                    ind_t = smp.tile([P, P], BF16, name="ind_t",
                                                     tag="ind_t")
                                    nc.vector.tensor_scalar(
                                        out=ind_t[:], in0=iota_b[:], scalar1=dcol,
                                        scalar2=None, op0=mybir.AluOpType.is_equal)
                                    nc.vector.tensor_tensor(
                                        out=w_b[:], in0=ind_t[:],
                                        in1=exps[:, ci * P:(ci + 1) * P],
                                        op=mybir.AluOpType.mult)
                                nc.tensor.matmul(pagg[:, :D], lhsT=w_b[:],
                                                 rhs=vg[:, c, :],
                                                 start=(c == 0), stop=(c == S - 1))
                                nc.tensor.matmul(pagg[:, D:D + 1], lhsT=w_b[:],
                                                 rhs=ones_v[:],
                                                 start=False, stop=(c == S - 1))
                    if layer >= 0:
                        for c in range(S):
                            dcol = dstc[:, t * S + c: t * S + c + 1]
                            # scatter TRANSPOSED: paggT_j[d_j, dst]
                            # accumulates in two psum halves (distinct banks).
                            ind_b = smp.tile([P, P], H_DT, name="ind_b", tag="w_b")
                            nc.vector.tensor_scalar(
                                out=ind_b[:], in0=iota_b[:], scalar1=dcol,
                                scalar2=None, op0=mybir.AluOpType.is_equal)
                            for j in range(DJ):
                                nc.tensor.matmul(
                                    sage_halves[j],
                                    lhsT=vg[:, c, j * P:(j + 1) * P],
                                    rhs=ind_b[:],
                                    start=(c == 0), stop=(c == S - 1))

                    # ---- tile epilogue -> h_out tile [node, d] ----
                    if layer < 0:
                        smax = smp.tile([P, 1], F32, name="smax")
                        nc.vector.tensor_scalar(
                            out=smax[:], in0=pagg[:, D:D + 1], scalar1=1e-30,
                            scalar2=None, op0=mybir.AluOpType.max)
                        rs = smp.tile([P, 1], F32, name="rs")
                        nc.vector.reciprocal(rs[:], smax[:])
                        pskip = ps.tile([P, D], F32, name="pskip", tag="pmm", bufs=kpmm)
                        for ji in range(DJ):
                            nc.tensor.matmul(pskip[:], lhsT=xtile(ji, t),
                                             rhs=wslice(WS, ji),
                                             start=(ji == 0), stop=False)
                        nc.tensor.matmul(pskip[:], lhsT=ones1[:],
                                         rhs=brow[0:1, :D],
                                         start=False, stop=True)
                        t1 = smp.tile([P, D], F32, name="t1", tag="t1")
                        nc.scalar.activation(t1[:], pagg[:, :D],
                                             mybir.ActivationFunctionType.Copy,
                                             scale=rs[:, :1])
                        t2 = smp.tile([P, D], F32, name="t2", tag="t2")
                        nc.vector.tensor_tensor(out=t2[:], in0=t1[:], in1=pskip[:],
                                                op=mybir.AluOpType.add)
                        nc.scalar.activation(h_out[:, t * D:(t + 1) * D], t2[:],
                                             mybir.ActivationFunctionType.Relu)
                    else:
                        # mean-term (gamma folded into Wl'): pzA = sumT^T @ Wl'
                        pzA = ps.tile([P, D], F32, name="pzA", tag="pmm", bufs=kpmm)
                        for j in range(DJ):
                            mT = smp.tile([P, P], D_DT, name="mT", tag="mT")
                            nc.scalar.copy(out=mT[:], in_=sage_halves[j])
                            nc.tensor.matmul(pzA[:], lhsT=mT[:],
                                             rhs=wslice(WL[layer], j),
                                             start=(j == 0), stop=(j == DJ - 1))
                        # gate(z) = al*z_affine + (1-al)h_prev: pzB already
                        # holds root+bias+residual; add mean term / deg.
                        t1 = smp.tile([P, D], F32, name="t1s", tag="t2")
                        nc.vector.tensor_scalar(
                            out=t1[:], in0=pzA[:], scalar1=invd[:, t:t + 1],
                            scalar2=None, op0=mybir.AluOpType.mult)
                        t2 = smp.tile([P, D], F32, name="t2s", tag="t3")
                        nc.vector.tensor_tensor(out=t2[:], in0=t1[:], in1=pzB[:],
                                                op=mybir.AluOpType.add)
                        if layer == L - 1:
                            houtf = smp.tile([P, D], F32, name="houtf", tag="t4")
                            nc.scalar.activation(houtf[:], t2[:],
                                                 mybir.ActivationFunctionType.Relu)
                            nc.sync.dma_start(out=out_dram[t * P:(t + 1) * P, :],
                                              in_=houtf[:])
                        else:
                            nc.scalar.activation(h_out[:, t * D:(t + 1) * D], t2[:],
                                                 mybir.ActivationFunctionType.Relu)

                    if layer < L - 1:
                        nc.sync.dma_start(out=hag_in[li][t * P:(t + 1) * P, :],
                                          in_=h_out[:, t * D:(t + 1) * D])
                        for j in range(DJ):
                            ptr2 = ps.tile([P, P], BF16, name="ptr2", tag="ptr", bufs=kptr)
                            nc.tensor.transpose(
                                out=ptr2[:],
                                in_=h_out[:, t * D + j * P: t * D + (j + 1) * P],
                                identity=ident_b[:])
                            nc.scalar.copy(
                                out=hT_out[:, j * sh + t * P: j * sh + (t + 1) * P],
                                in_=ptr2[:])

                if layer < L - 1:
                    allgather(hag_in[li], h_full[li])

            if stages <= 1:
                # dump k_full slice so the program has an output
                tmpo = smp.tile([P, D], F32, name="tmpo")
                for t in range(nt):
                    nc.vector.tensor_copy(out=tmpo[:], in_=xt[:, :D])
                    nc.sync.dma_start(out=out_dram[t * P:(t + 1) * P, :], in_=tmpo[:])
            else:
                agg_pass(-1, None, None, h_cur, hT_cur)
                bufs = [(h_cur, hT_cur), (h_nxt, hT_nxt)]
                for i in range(min(L, stages - 2)):
                    h_prev, hT_prev = bufs[i % 2]
                    h_out, hT_out = bufs[(i + 1) % 2]
                    agg_pass(i, h_prev, hT_prev, h_out, hT_out)
                if stages - 2 < L:
                    hsrc, _ = bufs[max(0, stages - 2) % 2]
                    for t in range(nt):
                        tmpo = smp.tile([P, D], F32, name="tmpo2")
                        nc.vector.tensor_copy(out=tmpo[:],
                                              in_=hsrc[:, t * D:(t + 1) * D])
                        nc.sync.dma_start(out=out_dram[t * P:(t + 1) * P, :],
                                          in_=tmpo[:])

    nc.compile()
    _nc_cache[key] = nc
    return nc


def _balance_perm(dst, n, n_pad):
    """Renumber nodes so each dst-tile of 128 has a near-equal edge count.
    Returns perm: old id -> new id (padding slots filled with virtual ids)."""
    import heapq

    deg = np.bincount(dst, minlength=n)
    nbins = n_pad // P
    counts = np.zeros(nbins, np.int64)
    perm = np.empty(n, np.int64)
    heap = [(0, g) for g in range(nbins)]
    heapq.heapify(heap)
    for node in np.argsort(-deg, kind="stable"):
        while True:
            load, g = heapq.heappop(heap)
            if counts[g] < P:
                break
        perm[node] = g * P + counts[g]
        counts[g] += 1
        heapq.heappush(heap, (load + int(deg[node]), g))
    return perm


def _host_prep(x, src, dst, Wq, bq, Wk, bk, Wv, bv, Ws, bs, Wl, bl, Wr,
               gamma, beta, alpha_res):
    n, d = x.shape
    n_pad = ((n + NC * P - 1) // (NC * P)) * (NC * P)
    sh = n_pad // NC
    nt = sh // P
    n_tiles = n_pad // P

    perm = _balance_perm(dst, n, n_pad)
    src = perm[src]
    dst = perm[dst]

    order = np.argsort(dst, kind="stable")
    src_s, dst_s = src[order], dst[order]
    tile_of = dst_s // P
    counts = np.bincount(tile_of, minlength=n_tiles)
    starts = np.concatenate([[0], np.cumsum(counts)])
    S = int(max(1, (counts.max() + P - 1) // P))
    ET = S * P

    deg = np.bincount(dst, minlength=n_pad).astype(np.float32)
    invdeg_full = 1.0 / np.maximum(deg, 1.0)

    al = 1.0 / (1.0 + np.exp(-alpha_res))
    oma = float(1.0 - al)
    bn_scale = 1.0 / np.sqrt(1.0 + BN_EPS)
    scale = 1.0 / np.sqrt(float(d))

    x_pad = np.zeros((n_pad, D), np.float32)
    x_pad[perm] = x
    xT = x_pad.T.copy()

    Gx = [al * bn_scale * gamma[i] for i in range(L)]
    Bx = [al * (bl[i] * bn_scale * gamma[i] + beta[i]) for i in range(L)]

    # gamma/bn/gate scale folded into the SAGE weights (per-output-column)
    weights = [Wq, Wk, Wv, Ws,
               Wl[0] * Gx[0][None, :], Wr[0] * Gx[0][None, :],
               Wl[1] * Gx[1][None, :], Wr[1] * Gx[1][None, :],
               Wl[2] * Gx[2][None, :], Wr[2] * Gx[2][None, :]]
    wpack = np.zeros((P, 11 * DJ * D), D_NP)
    for w, W in enumerate(weights):
        for j in range(DJ):
            wpack[:, (w * DJ + j) * D:(w * DJ + j + 1) * D] = W[j * P:(j + 1) * P, :]
    for j in range(DJ):
        blk = np.zeros((P, D), np.float32)
        blk[np.arange(P), j * P + np.arange(P)] = oma
        wpack[:, (10 * DJ + j) * D:(10 * DJ + j + 1) * D] = blk
    vecs = [bk, bv, bs, Gx[0], Bx[0], Gx[1], Bx[1], Gx[2], Bx[2]]
    vpack = np.empty((P, 9 * D + DJ), np.float32)
    for k, v in enumerate(vecs):
        vpack[:, k * D:(k + 1) * D] = np.tile(v[None, :], (P, 1))
    for j in range(DJ):
        vpack[:, 9 * D + j] = bq[j * P:(j + 1) * P]

    in_maps = []
    for r in range(NC):
        idx_arr = np.zeros((P, nt * S * 8), np.int16)
        dst_arr = np.full((P, nt * S), 128.0, np.float32)
        for tloc in range(nt):
            g = r * nt + tloc
            e0, e1 = starts[g], starts[g + 1]
            cnt = e1 - e0
            srcs = np.zeros(ET, np.int64)
            srcs[:cnt] = src_s[e0:e1]
            dl = np.full(ET, 128, np.int64)
            dl[:cnt] = dst_s[e0:e1] - g * P
            idx_arr[:, tloc * S * 8:(tloc + 1) * S * 8] = _wrap_idx(srcs)
            dst_arr[:, tloc * S:(tloc + 1) * S] = dl.reshape(S, P).T
        invdeg_r = invdeg_full[r * sh:(r + 1) * sh].reshape(nt, P).T.copy()

        xt_r = np.empty((P, DJ * sh), D_NP)
        for j in range(DJ):
            xt_r[:, j * sh:(j + 1) * sh] = xT[j * P:(j + 1) * P, r * sh:(r + 1) * sh]

        in_maps.append({
            "xt_in": xt_r,
            "wpack_in": wpack,
            "vpack_in": vpack,
            "idx_in": idx_arr,
            "dst_in": dst_arr,
            "invdeg_in": np.ascontiguousarray(invdeg_r),
        })
    return in_maps, (n_pad, sh, nt, S, scale, oma), perm


def kernel(**inputs):
    x = np.asarray(inputs["x"], np.float32)
    edge_index = np.asarray(inputs["edge_index"])
    args = dict(
        Wq=np.asarray(inputs["Wq"], np.float32), bq=np.asarray(inputs["bq"], np.float32),
        Wk=np.asarray(inputs["Wk"], np.float32), bk=np.asarray(inputs["bk"], np.float32),
        Wv=np.asarray(inputs["Wv"], np.float32), bv=np.asarray(inputs["bv"], np.float32),
        Ws=np.asarray(inputs["Ws"], np.float32), bs=np.asarray(inputs["bs"], np.float32),
        Wl=np.asarray(inputs["Wl"], np.float32), bl=np.asarray(inputs["bl"], np.float32),
        Wr=np.asarray(inputs["Wr"], np.float32),
        gamma=np.asarray(inputs["gamma"], np.float32),
        beta=np.asarray(inputs["beta"], np.float32),
        alpha_res=float(np.asarray(inputs["alpha_res"])),
    )
    src = edge_index[0].astype(np.int64)
    dst = edge_index[1].astype(np.int64)

    in_maps, (n_pad, sh, nt, S, scale, oma), perm = _host_prep(x, src, dst, **args)
    t0 = time.time()
    nc = build_nc(n_pad, sh, nt, S, scale, oma)
    print(f"[kernel] build+compile {time.time()-t0:.1f}s", flush=True)
    t0 = time.time()
    res = run_bass_kernel_spmd(nc, in_maps, core_ids=list(range(NC)))
    print(f"[kernel] run {time.time()-t0:.1f}s", flush=True)
    out = np.concatenate([res.results[r]["out"] for r in range(NC)], axis=0)
    return out[perm]



# revision 91
# speedup vs baseline: 1.0678x; 1.0551x over previous
"""Trainium2 Bass kernel for nn_MixGNN (TransformerConv + 3x SAGEConv + BN + gated residual).

Strategy (8 NeuronCores, dst-node sharding):
  - Pad N 10000 -> 10240; core r owns 1280 dst nodes = 10 tiles of 128.
  - Host prep: degree-balanced node renumbering (each dst-tile gets ~E/80
    edges, shrinking the padded chunk count S), edges sorted by dst and
    bucketed per tile, wrapped int16 gather indices, per-chunk local-dst
    columns, 1/deg, bf16-packed weights with BN gamma/gate folded into
    Wl/Wr columns and an extra oma*I block for the PE-side residual.
  - Device per layer: dense matmuls on PE with bf16 operands (1 cycle/row);
    per-edge work via dma_gather of source rows (512B descriptors) +
    indicator matmuls (Ind[e,n] = (dst_e==n) built by DVE is_equal).
    Transformer: scores KgT.T @ qT from a transposed bf16 gather, batched
    exp() over 4 score chunks per Act call, w_b = Ind*exp via one fused DVE
    op, agg + exp-sum accumulated in one PSUM group; epilogue adds x@Ws with
    the bias via a rank-1 ones-row matmul. SAGE: TRANSPOSED scatter
    (paggT_j[d_j,dst] in two distinct PSUM banks), pzA = sumT^T @ Wl' with
    1/deg applied after (commutes), and pzB = h@Wr' + bias + oma*h computed
    entirely on PE (bias via ones-row, residual via oma*I against hT) and
    issued before the gathers so PE fills the wait; epilogue is 2 DVE ops +
    relu, with non-final h kept bf16 so relu output feeds the AllGather
    staging and the hT transposes directly.
  - Halo exchange: AllGather of each core's h shard (bf16) into a full table
    in shared DRAM before every aggregation.
Output: fp32 [10000, 256].
"""
import os
import sys
import time

import numpy as np

for _p in ("/opt/trn_rl_repo",):
    if _p not in sys.path:
        sys.path.insert(0, _p)

import ml_dtypes  # noqa: E402
import concourse.bacc as bacc  # noqa: E402
import concourse.mybir as mybir  # noqa: E402
import concourse.tile as tile  # noqa: E402
from concourse.bass_utils import run_bass_kernel_spmd  # noqa: E402

P = 128
D = 256
DJ = D // P           # 2 d-chunks of 128
NC = 8                # cores
L = 3                 # SAGE layers
BN_EPS = 1e-5

# dtype knobs for gathered tables (accuracy vs bandwidth)
V_BF16 = True         # v table + attention agg in bf16
H_BF16 = True         # h tables + SAGE agg in bf16

F32 = mybir.dt.float32
F32R = (mybir.dt.float32 if os.environ.get("KNOF32R") == "1"
        else mybir.dt.float32r)  # f32 bits, PE full-rate mode at >=256 free
BF16 = mybir.dt.bfloat16
I16 = mybir.dt.int16
# dense-path operand dtype: bf16 halves DMA + SBUF for x/W/hT/mT tables
DENSE_BF16 = os.environ.get("KDENSE", "bf16") == "bf16"
D_DT = BF16 if DENSE_BF16 else F32R
D_NP = ml_dtypes.bfloat16 if DENSE_BF16 else np.float32
V_DT = BF16 if V_BF16 else F32
H_DT = BF16 if H_BF16 else F32

_nc_cache = {}


def _wrap_idx(a):
    """[S*128] int array -> [128, S*8] int16 wrapped gather-index layout."""
    w16 = a.reshape(-1, 16).T.astype(np.int16)   # [16, S*8]
    return np.tile(w16, (8, 1))                  # replicate to 8 Q7 stripes


def build_nc(n_pad, sh, nt, S, scale, oma):
    stages = int(os.environ.get("KSTAGES", "5"))
    nocc = os.environ.get("KNOCC") == "1"
    ksm = int(os.environ.get("KSM", "10"))
    kgp = int(os.environ.get("KGP", "2"))
    kpsc = int(os.environ.get("KPSC", "3"))
    kptr = int(os.environ.get("KPTR", "2"))
    kpagg = int(os.environ.get("KPAGG", "1"))
    kpmm = int(os.environ.get("KPMM", "2"))
    khalf = int(os.environ.get("KHALF", "4"))  # gather splits per tile
    kabl = os.environ.get("KABL", "")
    key = (n_pad, sh, nt, S, round(scale, 9), round(oma, 9), V_BF16, H_BF16, stages,
           nocc, ksm, kgp, kpsc, kptr, kpagg, kpmm, khalf, kabl,
           os.environ.get("KHALFT"),
           os.environ.get("KKGT"), os.environ.get("KVG"),
           os.environ.get("KEXP"), os.environ.get("KPOOLB"),
           os.environ.get("KWBSTT"), os.environ.get("KGRAD"))
    if key in _nc_cache:
        return _nc_cache[key]

    ET = S * P  # padded edges per tile
    ndev = 1 if nocc else NC
    nc = bacc.Bacc("TRN2", target_bir_lowering=False, debug=False, num_devices=ndev)

    xt_in = nc.dram_tensor("xt_in", [P, DJ * sh], D_DT, kind="ExternalInput")
    wpack_in = nc.dram_tensor("wpack_in", [P, 11 * DJ * D], D_DT, kind="ExternalInput")
    vpack_in = nc.dram_tensor("vpack_in", [P, 9 * D + DJ], F32, kind="ExternalInput")
    idx_in = nc.dram_tensor("idx_in", [P, nt * S * 8], I16, kind="ExternalInput")
    dst_in = nc.dram_tensor("dst_in", [P, nt * S], F32, kind="ExternalInput")
    invdeg_in = nc.dram_tensor("invdeg_in", [P, nt], F32, kind="ExternalInput")
    xfull_in = nc.dram_tensor("xfull_in", [n_pad, D], D_DT, kind="ExternalInput")
    out_dram = nc.dram_tensor("out", [sh, D], F32, kind="ExternalOutput")

    WQ, WK, WV, WS = 0, 1, 2, 3
    WL = [4, 6, 8]
    WR = [5, 7, 9]
    VBK, VBV, VBS = 0, 1, 2

    with tile.TileContext(nc) as tc:
        with (
            tc.tile_pool(name="cst", bufs=1) as cst,
            tc.tile_pool(name="sb", bufs=1) as sb,
            tc.tile_pool(name="g", bufs=kgp) as gp,
            tc.tile_pool(name="sm", bufs=ksm) as smp,
            tc.tile_pool(name="ps", bufs=2, space="PSUM") as ps,
            tc.tile_pool(name="dr", bufs=1, space="DRAM") as dr,
        ):
            # ---------------- constants / inputs to SBUF ----------------
            wp = cst.tile([P, 11 * DJ * D], D_DT)
            nc.sync.dma_start(out=wp[:], in_=wpack_in[:])
            vp = cst.tile([P, 9 * D + DJ], F32)
            nc.sync.dma_start(out=vp[:], in_=vpack_in[:])
            xt = cst.tile([P, DJ * sh], D_DT)
            for _xi in range(4):
                _c0 = _xi * (DJ * sh // 4)
                _c1 = (_xi + 1) * (DJ * sh // 4)
                nc.sync.dma_start(out=xt[:, _c0:_c1], in_=xt_in[:, _c0:_c1])
            dstc = cst.tile([P, nt * S], F32)
            nc.sync.dma_start(out=dstc[:], in_=dst_in[:])
            invd = cst.tile([P, nt], F32)
            nc.sync.dma_start(out=invd[:], in_=invdeg_in[:])
            idx_sb = cst.tile([P, nt * S * 8], I16)
            nc.sync.dma_start(out=idx_sb[:], in_=idx_in[:])

            iota_i = cst.tile([P, P], mybir.dt.int32)
            nc.gpsimd.iota(iota_i[:], pattern=[[1, P]], base=0, channel_multiplier=0)
            ones_v = cst.tile([P, 1], V_DT)
            nc.vector.memset(ones_v[:], 1.0)
            # identity for PE transposes: (iota_row == partition_idx)
            iota_part = cst.tile([P, 1], mybir.dt.int32)
            nc.gpsimd.iota(iota_part[:], pattern=[[1, 1]], base=0, channel_multiplier=1)
            iota_part_f = cst.tile([P, 1], F32)
            nc.vector.tensor_copy(out=iota_part_f[:], in_=iota_part[:])
            iota_b = cst.tile([P, P], BF16)
            nc.vector.tensor_copy(out=iota_b[:], in_=iota_i[:])
            ident_b = cst.tile([P, P], BF16)
            nc.vector.tensor_scalar(
                out=ident_b[:], in0=iota_b[:], scalar1=iota_part_f[:, :1],
                scalar2=None, op0=mybir.AluOpType.is_equal,
            )
            # rank-1 bias rows + oma*I blocks (bias/residual folded into PE)
            ones1 = cst.tile([1, P], BF16)
            nc.vector.memset(ones1[:], 1.0)
            brow = cst.tile([1, (L + 1) * D], BF16)
            nc.vector.tensor_copy(out=brow[0:1, :D], in_=vp[0:1, 2 * D:3 * D])
            for _i in range(L):
                nc.vector.tensor_copy(
                    out=brow[0:1, (_i + 1) * D:(_i + 2) * D],
                    in_=vp[0:1, (4 + 2 * _i) * D:(5 + 2 * _i) * D])
            omaI = wp[:, 10 * DJ * D:11 * DJ * D]  # host-packed oma*I blocks

            def wslice(w, j):
                return wp[:, (w * DJ + j) * D:(w * DJ + j + 1) * D]

            def vslice(k):
                return vp[:, k * D:(k + 1) * D]

            def xtile(j, t):
                return xt[:, j * sh + t * P: j * sh + (t + 1) * P]

            # ---------------- DRAM tables ----------------
            v_ag_in = dr.tile([sh, D], V_DT)
            v_full = dr.tile([n_pad, D], V_DT, addr_space="Shared")
            hag_in = [dr.tile([sh, D], H_DT, name=f"hag_in_{i}") for i in range(L)]
            h_full = [dr.tile([n_pad, D], H_DT, name=f"h_full_{i}", addr_space="Shared")
                      for i in range(L)]

            def allgather(in_t, out_t):
                if nocc:
                    nc.sync.dma_start(out=out_t[:sh], in_=in_t[:])
                else:
                    nc.gpsimd.collective_compute(
                        "AllGather", mybir.AluOpType.bypass,
                        replica_groups=[list(range(NC))],
                        ins=[in_t[:]], outs=[out_t[:]],
                    )

            # ---------------- stage 0: k,v shard tables + AG, then qT ----------------
            for t in range(nt):
                pv = ps.tile([P, D], F32, name="pv", tag="pmm", bufs=kpmm)
                for ji in range(DJ):
                    nc.tensor.matmul(pv[:], lhsT=xtile(ji, t), rhs=wslice(WV, ji),
                                     start=(ji == 0), stop=(ji == DJ - 1))
                v_sb = smp.tile([P, D], V_DT, name="v_sb")
                nc.vector.tensor_tensor(out=v_sb[:], in0=pv[:], in1=vslice(VBV),
                                        op=mybir.AluOpType.add)
                nc.sync.dma_start(out=v_ag_in[t * P:(t + 1) * P, :], in_=v_sb[:])

            allgather(v_ag_in, v_full)

            qT = []
            for j in range(DJ):
                qTj = sb.tile([P, sh], BF16, name=f"qT_{j}")
                n0 = 0
                while n0 < sh:
                    nn = min(512, sh - n0)
                    pq = ps.tile([P, 512], F32, name="pq", tag="pmm", bufs=kpmm)
                    for ji in range(DJ):
                        nc.tensor.matmul(
                            pq[:, :nn],
                            lhsT=wslice(WQ, ji)[:, j * P:(j + 1) * P],
                            rhs=xt[:, ji * sh + n0: ji * sh + n0 + nn],
                            start=(ji == 0), stop=(ji == DJ - 1),
                        )
                    nc.vector.tensor_scalar(
                        out=qTj[:, n0:n0 + nn], in0=pq[:, :nn],
                        scalar1=vp[:, 9 * D + j: 9 * D + j + 1], scalar2=None,
                        op0=mybir.AluOpType.add,
                    )
                    n0 += nn
                qT.append(qTj)

            # shard-resident activations
            h_cur = sb.tile([P, nt * D], H_DT)
            h_nxt = sb.tile([P, nt * D], H_DT)
            hT_cur = sb.tile([P, DJ * sh], D_DT)
            hT_nxt = sb.tile([P, DJ * sh], D_DT)

            def agg_pass(layer, h_prev, hT_prev, h_out, hT_out):
                """layer -1: transformer (h_prev/hT_prev unused); 0..L-1: SAGE."""
                li = layer + 1  # h table index this pass WRITES (0 for transformer)
                kh = khalf if layer >= 0 else int(os.environ.get("KHALFT", "3"))
                splits = []  # (c0, c1) chunk ranges per gather piece
                base = (S + kh - 1) // kh
                c0 = 0
                while c0 < S:
                    splits.append((c0, min(S, c0 + base)))
                    c0 += base
                # graded head for tile 0: a small leading split lets chunk-0
                # consumers start right after the pass-gating AllGather
                kgrad = int(os.environ.get("KGRAD", "0"))
                splits0 = splits
                if kgrad and layer >= 0:
                    splits0 = [(0, kgrad)]
                    c0 = kgrad
                    while c0 < S:
                        splits0.append((c0, min(S, c0 + base)))
                        c0 += base
                for t in range(nt):
                    if layer < 0:
                        kgt = gp.tile([P, DJ, ET], BF16, name="kgt", tag="kgt",
                                      bufs=int(os.environ.get("KKGT", "2")))
                        vg = gp.tile([P, S, D], V_DT, name="vg", tag="vg",
                                     bufs=int(os.environ.get("KVG", "3")))
                        pzB = None
                    else:
                        kgt = None
                        vg = gp.tile([P, S, D], H_DT, name="hg", tag="vg",
                                     bufs=int(os.environ.get("KVG", "3")))
                        # root + bias + gated residual, all on PE — independent
                        # of the gather; issue first so PE fills the wait.
                        pzB = ps.tile([P, D], F32, name="pzB", tag="pmm", bufs=kpmm)
                        for j in range(DJ):
                            nc.tensor.matmul(
                                pzB[:],
                                lhsT=hT_prev[:, j * sh + t * P: j * sh + (t + 1) * P],
                                rhs=wslice(WR[layer], j),
                                start=(j == 0), stop=False)
                        nc.tensor.matmul(
                            pzB[:], lhsT=ones1[:],
                            rhs=brow[0:1, (layer + 1) * D:(layer + 2) * D],
                            start=False, stop=False)
                        for j in range(DJ):
                            nc.tensor.matmul(
                                pzB[:],
                                lhsT=hT_prev[:, j * sh + t * P: j * sh + (t + 1) * P],
                                rhs=omaI[:, j * D:(j + 1) * D],
                                start=False, stop=(j == DJ - 1))
                    if layer < 0:
                        idx_tt = idx_sb[:, t * S * 8:(t + 1) * S * 8]
                        nc.gpsimd.dma_gather(
                            out_ap=kgt[:], in_ap=xfull_in[:], idxs_ap=idx_tt,
                            num_idxs=ET, num_idxs_reg=ET, elem_size=D,
                            transpose=True, single_packet=False)
                    src_tab = v_full if layer < 0 else h_full[layer]
                    for (ca, cb) in (splits0 if t == 0 else splits):
                        nn_i = (cb - ca) * P
                        idx_t = idx_sb[:, t * S * 8 + ca * 8: t * S * 8 + cb * 8]
                        nc.gpsimd.dma_gather(
                            out_ap=vg[:, ca:cb, :], in_ap=src_tab[:], idxs_ap=idx_t,
                            num_idxs=nn_i, num_idxs_reg=nn_i, elem_size=D,
                            single_packet=False)

                    pagg = ps.tile([P, D + 1], F32, name="pagg", tag="pagg", bufs=kpagg)
                    if layer >= 0:
                        # two transposed-scatter accumulators must sit in
                        # DIFFERENT psum banks (per-bank accumulation state)
                        pagg2 = ps.tile([P, P], F32, name="pagg2", tag="psc", bufs=kpsc)
                        sage_halves = [pagg[:, :P], pagg2[:]]
                    else:
                        sage_halves = None
                    kexp = int(os.environ.get("KEXP", "4"))  # chunks per exp call
                    # walrus rejects TensorScalarPtr on Pool; keep builds on DVE
                    kpool = int(os.environ.get("KPOOLB", "0"))
                    if layer < 0:
                        # batched scores: groups of kexp chunks share one psum
                        # bank and one exp() call (amortizes Act access lat.)
                        for c0 in range(0, S, kexp):
                            cn = min(kexp, S - c0)
                            pscw = ps.tile([P, kexp * P], F32, name="pscw",
                                           tag="psc", bufs=kpsc)
                            for ci in range(cn):
                                c = c0 + ci
                                for j in range(DJ):
                                    nc.tensor.matmul(
                                        pscw[:, ci * P:(ci + 1) * P],
                                        lhsT=kgt[:, j, c * P:(c + 1) * P],
                                        rhs=qT[j][:, t * P:(t + 1) * P],
                                        start=(j == 0), stop=(j == DJ - 1))
                            exps = smp.tile([P, kexp * P], BF16, name="exps")
                            nc.scalar.activation(exps[:, :cn * P], pscw[:, :cn * P],
                                                 mybir.ActivationFunctionType.Exp,
                                                 scale=scale)
                            for ci in range(cn):
                                c = c0 + ci
                                dcol = dstc[:, t * S + c: t * S + c + 1]
                                w_b = smp.tile([P, P], V_DT, name="w_b", tag="w_b")
                                if os.environ.get("KWBSTT", "1") == "1":
                                    nc.vector.scalar_tensor_tensor(
                                        out=w_b[:], in0=iota_b[:], scalar=dcol,
                                        in1=exps[:, ci * P:(ci + 1) * P],
                                        op0=mybir.AluOpType.is_equal,
                                        op1=mybir.AluOpType.mult)
                                else:
                                    # two simple all-bf16 ops hit the DVE 2x
                                    # path; the fused STT variant does not
                                    ind_t = smp.tile([P, P], BF16, name="ind_t",
                                                     tag="ind_t")
                                    nc.vector.tensor_scalar(
                                        out=ind_t[:], in0=iota_b[:], scalar1=dcol,
                                        scalar2=None, op0=mybir.AluOpType.is_equal)
                                    nc.vector.tensor_tensor(
                                        out=w_b[:], in0=ind_t[:],
                                        in1=exps[:, ci * P:(ci + 1) * P],
                                        op=mybir.AluOpType.mult)
                                nc.tensor.matmul(pagg[:, :D], lhsT=w_b[:],
                                                 rhs=vg[:, c, :],
                                                 start=(c == 0), stop=(c == S - 1))
                                nc.tensor.matmul(pagg[:, D:D + 1], lhsT=w_b[:],
                                                 rhs=ones_v[:],
                                                 start=False, stop=(c == S - 1))
                    if layer >= 0:
                        for c in range(S):
                            dcol = dstc[:, t * S + c: t * S + c + 1]
                            # scatter TRANSPOSED: paggT_j[d_j, dst]
                            # accumulates in two psum halves (distinct banks).
                            ind_b = smp.tile([P, P], H_DT, name="ind_b", tag="w_b")
                            nc.vector.tensor_scalar(
                                out=ind_b[:], in0=iota_b[:], scalar1=dcol,
                                scalar2=None, op0=mybir.AluOpType.is_equal)
                            for j in range(DJ):
                                nc.tensor.matmul(
                                    sage_halves[j],
                                    lhsT=vg[:, c, j * P:(j + 1) * P],
                                    rhs=ind_b[:],
                                    start=(c == 0), stop=(c == S - 1))

                    # ---- tile epilogue -> h_out tile [node, d] ----
                    if layer < 0:
                        smax = smp.tile([P, 1], F32, name="smax")
                        nc.vector.tensor_scalar(
                            out=smax[:], in0=pagg[:, D:D + 1], scalar1=1e-30,
                            scalar2=None, op0=mybir.AluOpType.max)
                        rs = smp.tile([P, 1], F32, name="rs")
                        nc.vector.reciprocal(rs[:], smax[:])
                        pskip = ps.tile([P, D], F32, name="pskip", tag="pmm", bufs=kpmm)
                        for ji in range(DJ):
                            nc.tensor.matmul(pskip[:], lhsT=xtile(ji, t),
                                             rhs=wslice(WS, ji),
                                             start=(ji == 0), stop=False)
                        nc.tensor.matmul(pskip[:], lhsT=ones1[:],
                                         rhs=brow[0:1, :D],
                                         start=False, stop=True)
                        t1 = smp.tile([P, D], F32, name="t1", tag="t1")
                        nc.scalar.activation(t1[:], pagg[:, :D],
                                             mybir.ActivationFunctionType.Copy,
                                             scale=rs[:, :1])
                        t2 = smp.tile([P, D], F32, name="t2", tag="t2")
                        nc.vector.tensor_tensor(out=t2[:], in0=t1[:], in1=pskip[:],
                                                op=mybir.AluOpType.add)
                        nc.scalar.activation(h_out[:, t * D:(t + 1) * D], t2[:],
                                             mybir.ActivationFunctionType.Relu)
                    else:
                        # mean-term (gamma folded into Wl'): pzA = sumT^T @ Wl'
                        pzA = ps.tile([P, D], F32, name="pzA", tag="pmm", bufs=kpmm)
                        for j in range(DJ):
                            mT = smp.tile([P, P], D_DT, name="mT", tag="mT")
                            nc.scalar.copy(out=mT[:], in_=sage_halves[j])
                            nc.tensor.matmul(pzA[:], lhsT=mT[:],
                                             rhs=wslice(WL[layer], j),
                                             start=(j == 0), stop=(j == DJ - 1))
                        # gate(z) = al*z_affine + (1-al)h_prev: pzB already
                        # holds root+bias+residual; add mean term / deg.
                        t1 = smp.tile([P, D], F32, name="t1s", tag="t2")
                        nc.vector.tensor_scalar(
                            out=t1[:], in0=pzA[:], scalar1=invd[:, t:t + 1],
                            scalar2=None, op0=mybir.AluOpType.mult)
                        t2 = smp.tile([P, D], F32, name="t2s", tag="t3")
                        nc.vector.tensor_tensor(out=t2[:], in0=t1[:], in1=pzB[:],
                                                op=mybir.AluOpType.add)
                        if layer == L - 1:
                            houtf = smp.tile([P, D], F32, name="houtf", tag="t4")
                            nc.scalar.activation(houtf[:], t2[:],
                                                 mybir.ActivationFunctionType.Relu)
                            nc.sync.dma_start(out=out_dram[t * P:(t + 1) * P, :],
                                              in_=houtf[:])
                        else:
                            nc.scalar.activation(h_out[:, t * D:(t + 1) * D], t2[:],
                                                 mybir.ActivationFunctionType.Relu)

                    if layer < L - 1:
                        nc.sync.dma_start(out=hag_in[li][t * P:(t + 1) * P, :],
                                          in_=h_out[:, t * D:(t + 1) * D])
                        for j in range(DJ):
                            ptr2 = ps.tile([P, P], BF16, name="ptr2", tag="ptr", bufs=kptr)
                            nc.tensor.transpose(
                                out=ptr2[:],
                                in_=h_out[:, t * D + j * P: t * D + (j + 1) * P],
                                identity=ident_b[:])
                            nc.scalar.copy(
                                out=hT_out[:, j * sh + t * P: j * sh + (t + 1) * P],
                                in_=ptr2[:])

                if layer < L - 1:
                    allgather(hag_in[li], h_full[li])

            if stages <= 1:
                # dump k_full slice so the program has an output
                tmpo = smp.tile([P, D], F32, name="tmpo")
                for t in range(nt):
                    nc.vector.tensor_copy(out=tmpo[:], in_=xt[:, :D])
                    nc.sync.dma_start(out=out_dram[t * P:(t + 1) * P, :], in_=tmpo[:])
            else:
                agg_pass(-1, None, None, h_cur, hT_cur)
                bufs = [(h_cur, hT_cur), (h_nxt, hT_nxt)]
                for i in range(min(L, stages - 2)):
                    h_prev, hT_prev = bufs[i % 2]
                    h_out, hT_out = bufs[(i + 1) % 2]
                    agg_pass(i, h_prev, hT_prev, h_out, hT_out)
                if stages - 2 < L:
                    hsrc, _ = bufs[max(0, stages - 2) % 2]
                    for t in range(nt):
                        tmpo = smp.tile([P, D], F32, name="tmpo2")
                        nc.vector.tensor_copy(out=tmpo[:],
                                              in_=hsrc[:, t * D:(t + 1) * D])
                        nc.sync.dma_start(out=out_dram[t * P:(t + 1) * P, :],
                                          in_=tmpo[:])

    nc.compile()
    _nc_cache[key] = nc
    return nc


def _balance_perm(dst, n, n_pad):
    """Renumber nodes so each dst-tile of 128 has a near-equal edge count.
    Returns perm: old id -> new id (padding slots filled with virtual ids)."""
    import heapq

    deg = np.bincount(dst, minlength=n)
    nbins = n_pad // P
    counts = np.zeros(nbins, np.int64)
    perm = np.empty(n, np.int64)
    heap = [(0, g) for g in range(nbins)]
    heapq.heapify(heap)
    for node in np.argsort(-deg, kind="stable"):
        while True:
            load, g = heapq.heappop(heap)
            if counts[g] < P:
                break
        perm[node] = g * P + counts[g]
        counts[g] += 1
        heapq.heappush(heap, (load + int(deg[node]), g))
    return perm


def _host_prep(x, src, dst, Wq, bq, Wk, bk, Wv, bv, Ws, bs, Wl, bl, Wr,
               gamma, beta, alpha_res):
    n, d = x.shape
    n_pad = ((n + NC * P - 1) // (NC * P)) * (NC * P)
    sh = n_pad // NC
    nt = sh // P
    n_tiles = n_pad // P

    perm = _balance_perm(dst, n, n_pad)
    src = perm[src]
    dst = perm[dst]

    order = np.argsort(dst, kind="stable")
    src_s, dst_s = src[order], dst[order]
    tile_of = dst_s // P
    counts = np.bincount(tile_of, minlength=n_tiles)
    starts = np.concatenate([[0], np.cumsum(counts)])
    S = int(max(1, (counts.max() + P - 1) // P))
    ET = S * P

    deg = np.bincount(dst, minlength=n_pad).astype(np.float32)
    invdeg_full = 1.0 / np.maximum(deg, 1.0)

    al = 1.0 / (1.0 + np.exp(-alpha_res))
    oma = float(1.0 - al)
    bn_scale = 1.0 / np.sqrt(1.0 + BN_EPS)
    scale = 1.0 / np.sqrt(float(d))

    x_pad = np.zeros((n_pad, D), np.float32)
    x_pad[perm] = x
    xT = x_pad.T.copy()
    x_full_b = x_pad.astype(ml_dtypes.bfloat16)

    Gx = [al * bn_scale * gamma[i] for i in range(L)]
    Bx = [al * (bl[i] * bn_scale * gamma[i] + beta[i]) for i in range(L)]

    Wq_eff = (Wq.astype(np.float64) @ Wk.astype(np.float64).T).astype(np.float32)
    bq_eff = (bq.astype(np.float64) @ Wk.astype(np.float64).T).astype(np.float32)

    # gamma/bn/gate scale folded into the SAGE weights (per-output-column)
    weights = [Wq_eff, Wk, Wv, Ws,
               Wl[0] * Gx[0][None, :], Wr[0] * Gx[0][None, :],
               Wl[1] * Gx[1][None, :], Wr[1] * Gx[1][None, :],
               Wl[2] * Gx[2][None, :], Wr[2] * Gx[2][None, :]]
    wpack = np.zeros((P, 11 * DJ * D), D_NP)
    for w, W in enumerate(weights):
        for j in range(DJ):
            wpack[:, (w * DJ + j) * D:(w * DJ + j + 1) * D] = W[j * P:(j + 1) * P, :]
    for j in range(DJ):
        blk = np.zeros((P, D), np.float32)
        blk[np.arange(P), j * P + np.arange(P)] = oma
        wpack[:, (10 * DJ + j) * D:(10 * DJ + j + 1) * D] = blk
    vecs = [bk, bv, bs, Gx[0], Bx[0], Gx[1], Bx[1], Gx[2], Bx[2]]
    vpack = np.empty((P, 9 * D + DJ), np.float32)
    for k, v in enumerate(vecs):
        vpack[:, k * D:(k + 1) * D] = np.tile(v[None, :], (P, 1))
    for j in range(DJ):
        vpack[:, 9 * D + j] = bq_eff[j * P:(j + 1) * P]

    in_maps = []
    for r in range(NC):
        idx_arr = np.zeros((P, nt * S * 8), np.int16)
        dst_arr = np.full((P, nt * S), 128.0, np.float32)
        for tloc in range(nt):
            g = r * nt + tloc
            e0, e1 = starts[g], starts[g + 1]
            cnt = e1 - e0
            srcs = np.zeros(ET, np.int64)
            srcs[:cnt] = src_s[e0:e1]
            dl = np.full(ET, 128, np.int64)
            dl[:cnt] = dst_s[e0:e1] - g * P
            idx_arr[:, tloc * S * 8:(tloc + 1) * S * 8] = _wrap_idx(srcs)
            dst_arr[:, tloc * S:(tloc + 1) * S] = dl.reshape(S, P).T
        invdeg_r = invdeg_full[r * sh:(r + 1) * sh].reshape(nt, P).T.copy()

        xt_r = np.empty((P, DJ * sh), D_NP)
        for j in range(DJ):
            xt_r[:, j * sh:(j + 1) * sh] = xT[j * P:(j + 1) * P, r * sh:(r + 1) * sh]

        in_maps.append({
            "xt_in": xt_r,
            "xfull_in": x_full_b,
            "wpack_in": wpack,
            "vpack_in": vpack,
            "idx_in": idx_arr,
            "dst_in": dst_arr,
            "invdeg_in": np.ascontiguousarray(invdeg_r),
        })
    return in_maps, (n_pad, sh, nt, S, scale, oma), perm


def kernel(**inputs):
    x = np.asarray(inputs["x"], np.float32)
    edge_index = np.asarray(inputs["edge_index"])
    args = dict(
        Wq=np.asarray(inputs["Wq"], np.float32), bq=np.asarray(inputs["bq"], np.float32),
        Wk=np.asarray(inputs["Wk"], np.float32), bk=np.asarray(inputs["bk"], np.float32),
        Wv=np.asarray(inputs["Wv"], np.float32), bv=np.asarray(inputs["bv"], np.float32),
        Ws=np.asarray(inputs["Ws"], np.float32), bs=np.asarray(inputs["bs"], np.float32),
        Wl=np.asarray(inputs["Wl"], np.float32), bl=np.asarray(inputs["bl"], np.float32),
        Wr=np.asarray(inputs["Wr"], np.float32),
        gamma=np.asarray(inputs["gamma"], np.float32),
        beta=np.asarray(inputs["beta"], np.float32),
        alpha_res=float(np.asarray(inputs["alpha_res"])),
    )
    src = edge_index[0].astype(np.int64)
    dst = edge_index[1].astype(np.int64)

    in_maps, (n_pad, sh, nt, S, scale, oma), perm = _host_prep(x, src, dst, **args)
    t0 = time.time()
    nc = build_nc(n_pad, sh, nt, S, scale, oma)
    print(f"[kernel] build+compile {time.time()-t0:.1f}s", flush=True)
    t0 = time.time()
    res = run_bass_kernel_spmd(nc, in_maps, core_ids=list(range(NC)))
    print(f"[kernel] run {time.time()-t0:.1f}s", flush=True)
    out = np.concatenate([res.results[r]["out"] for r in range(NC)], axis=0)
    return out[perm]



# revision 92
# speedup vs baseline: 1.0977x; 1.0279x over previous
"""Trainium2 Bass kernel for nn_MixGNN (TransformerConv + 3x SAGEConv + BN + gated residual).

Strategy (8 NeuronCores, dst-node sharding):
  - Pad N 10000 -> 10240; core r owns 1280 dst nodes = 10 tiles of 128.
  - Host prep: degree-balanced node renumbering (each dst-tile gets ~E/80
    edges, shrinking the padded chunk count S), edges sorted by dst and
    bucketed per tile, wrapped int16 gather indices, per-chunk local-dst
    columns, 1/deg, bf16-packed weights with BN gamma/gate folded into
    Wl/Wr columns and an extra oma*I block for the PE-side residual.
  - Device per layer: dense matmuls on PE with bf16 operands (1 cycle/row);
    per-edge work via dma_gather of source rows (512B descriptors) +
    indicator matmuls (Ind[e,n] = (dst_e==n) built by DVE is_equal).
    Transformer: scores KgT.T @ qT from a transposed bf16 gather, batched
    exp() over 4 score chunks per Act call, w_b = Ind*exp via one fused DVE
    op, agg + exp-sum accumulated in one PSUM group; epilogue adds x@Ws with
    the bias via a rank-1 ones-row matmul. SAGE: TRANSPOSED scatter
    (paggT_j[d_j,dst] in two distinct PSUM banks), pzA = sumT^T @ Wl' with
    1/deg applied after (commutes), and pzB = h@Wr' + bias + oma*h computed
    entirely on PE (bias via ones-row, residual via oma*I against hT) and
    issued before the gathers so PE fills the wait; epilogue is 2 DVE ops +
    relu, with non-final h kept bf16 so relu output feeds the AllGather
    staging and the hT transposes directly.
  - Halo exchange: AllGather of each core's h shard (bf16) into a full table
    in shared DRAM before every aggregation.
Output: fp32 [10000, 256].
"""
import os
import sys
import time

import numpy as np

for _p in ("/opt/trn_rl_repo",):
    if _p not in sys.path:
        sys.path.insert(0, _p)

import ml_dtypes  # noqa: E402
import concourse.bacc as bacc  # noqa: E402
import concourse.mybir as mybir  # noqa: E402
import concourse.tile as tile  # noqa: E402
from concourse.bass_utils import run_bass_kernel_spmd  # noqa: E402

P = 128
D = 256
DJ = D // P           # 2 d-chunks of 128
NC = 8                # cores
L = 3                 # SAGE layers
BN_EPS = 1e-5

# dtype knobs for gathered tables (accuracy vs bandwidth)
V_BF16 = True         # v table + attention agg in bf16
H_BF16 = True         # h tables + SAGE agg in bf16

F32 = mybir.dt.float32
F32R = (mybir.dt.float32 if os.environ.get("KNOF32R") == "1"
        else mybir.dt.float32r)  # f32 bits, PE full-rate mode at >=256 free
BF16 = mybir.dt.bfloat16
I16 = mybir.dt.int16
# dense-path operand dtype: bf16 halves DMA + SBUF for x/W/hT/mT tables
DENSE_BF16 = os.environ.get("KDENSE", "bf16") == "bf16"
D_DT = BF16 if DENSE_BF16 else F32R
D_NP = ml_dtypes.bfloat16 if DENSE_BF16 else np.float32
V_DT = BF16 if V_BF16 else F32
H_DT = BF16 if H_BF16 else F32

_nc_cache = {}


def _wrap_idx(a):
    """[S*128] int array -> [128, S*8] int16 wrapped gather-index layout."""
    w16 = a.reshape(-1, 16).T.astype(np.int16)   # [16, S*8]
    return np.tile(w16, (8, 1))                  # replicate to 8 Q7 stripes


def build_nc(n_pad, sh, nt, S, scale, oma):
    stages = int(os.environ.get("KSTAGES", "5"))
    nocc = os.environ.get("KNOCC") == "1"
    ksm = int(os.environ.get("KSM", "10"))
    kgp = int(os.environ.get("KGP", "2"))
    kpsc = int(os.environ.get("KPSC", "3"))
    kptr = int(os.environ.get("KPTR", "2"))
    kpagg = int(os.environ.get("KPAGG", "1"))
    kpmm = int(os.environ.get("KPMM", "2"))
    khalf = int(os.environ.get("KHALF", "4"))  # gather splits per tile
    kabl = os.environ.get("KABL", "")
    key = (n_pad, sh, nt, S, round(scale, 9), round(oma, 9), V_BF16, H_BF16, stages,
           nocc, ksm, kgp, kpsc, kptr, kpagg, kpmm, khalf, kabl,
           os.environ.get("KHALFT"),
           os.environ.get("KKGT"), os.environ.get("KVG"),
           os.environ.get("KEXP"), os.environ.get("KPOOLB"),
           os.environ.get("KWBSTT"), os.environ.get("KGRAD"))
    if key in _nc_cache:
        return _nc_cache[key]

    ET = S * P  # padded edges per tile
    ndev = 1 if nocc else NC
    nc = bacc.Bacc("TRN2", target_bir_lowering=False, debug=False, num_devices=ndev)

    xt_in = nc.dram_tensor("xt_in", [P, DJ * sh], D_DT, kind="ExternalInput")
    wpack_in = nc.dram_tensor("wpack_in", [P, 11 * DJ * D], D_DT, kind="ExternalInput")
    vpack_in = nc.dram_tensor("vpack_in", [P, 9 * D + DJ], F32, kind="ExternalInput")
    idx_in = nc.dram_tensor("idx_in", [P, nt * S * 8], I16, kind="ExternalInput")
    dst_in = nc.dram_tensor("dst_in", [P, nt * S], F32, kind="ExternalInput")
    invdeg_in = nc.dram_tensor("invdeg_in", [P, nt], F32, kind="ExternalInput")
    xfull_in = nc.dram_tensor("xfull_in", [n_pad, D], D_DT, kind="ExternalInput")
    out_dram = nc.dram_tensor("out", [sh, D], F32, kind="ExternalOutput")

    WQ, WK, WV, WS = 0, 1, 2, 3
    WL = [4, 6, 8]
    WR = [5, 7, 9]
    VBK, VBV, VBS = 0, 1, 2

    with tile.TileContext(nc) as tc:
        with (
            tc.tile_pool(name="cst", bufs=1) as cst,
            tc.tile_pool(name="sb", bufs=1) as sb,
            tc.tile_pool(name="g", bufs=kgp) as gp,
            tc.tile_pool(name="sm", bufs=ksm) as smp,
            tc.tile_pool(name="ps", bufs=2, space="PSUM") as ps,
            tc.tile_pool(name="dr", bufs=1, space="DRAM") as dr,
        ):
            # ---------------- constants / inputs to SBUF ----------------
            wp = cst.tile([P, 11 * DJ * D], D_DT)
            nc.sync.dma_start(out=wp[:], in_=wpack_in[:])
            vp = cst.tile([P, 9 * D + DJ], F32)
            nc.sync.dma_start(out=vp[:], in_=vpack_in[:])
            xt = cst.tile([P, DJ * sh], D_DT)
            for _xi in range(4):
                _c0 = _xi * (DJ * sh // 4)
                _c1 = (_xi + 1) * (DJ * sh // 4)
                nc.sync.dma_start(out=xt[:, _c0:_c1], in_=xt_in[:, _c0:_c1])
            dstc = cst.tile([P, nt * S], F32)
            nc.sync.dma_start(out=dstc[:], in_=dst_in[:])
            invd = cst.tile([P, nt], F32)
            nc.sync.dma_start(out=invd[:], in_=invdeg_in[:])
            idx_sb = cst.tile([P, nt * S * 8], I16)
            nc.sync.dma_start(out=idx_sb[:], in_=idx_in[:])

            iota_i = cst.tile([P, P], mybir.dt.int32)
            nc.gpsimd.iota(iota_i[:], pattern=[[1, P]], base=0, channel_multiplier=0)
            ones_v = cst.tile([P, 1], V_DT)
            nc.vector.memset(ones_v[:], 1.0)
            # identity for PE transposes: (iota_row == partition_idx)
            iota_part = cst.tile([P, 1], mybir.dt.int32)
            nc.gpsimd.iota(iota_part[:], pattern=[[1, 1]], base=0, channel_multiplier=1)
            iota_part_f = cst.tile([P, 1], F32)
            nc.vector.tensor_copy(out=iota_part_f[:], in_=iota_part[:])
            iota_b = cst.tile([P, P], BF16)
            nc.vector.tensor_copy(out=iota_b[:], in_=iota_i[:])
            ident_b = cst.tile([P, P], BF16)
            nc.vector.tensor_scalar(
                out=ident_b[:], in0=iota_b[:], scalar1=iota_part_f[:, :1],
                scalar2=None, op0=mybir.AluOpType.is_equal,
            )
            # rank-1 bias rows + oma*I blocks (bias/residual folded into PE)
            ones1 = cst.tile([1, P], BF16)
            nc.vector.memset(ones1[:], 1.0)
            brow = cst.tile([1, (L + 1) * D], BF16)
            nc.vector.tensor_copy(out=brow[0:1, :D], in_=vp[0:1, 2 * D:3 * D])
            for _i in range(L):
                nc.vector.tensor_copy(
                    out=brow[0:1, (_i + 1) * D:(_i + 2) * D],
                    in_=vp[0:1, (4 + 2 * _i) * D:(5 + 2 * _i) * D])
            omaI = wp[:, 10 * DJ * D:11 * DJ * D]  # host-packed oma*I blocks

            def wslice(w, j):
                return wp[:, (w * DJ + j) * D:(w * DJ + j + 1) * D]

            def vslice(k):
                return vp[:, k * D:(k + 1) * D]

            def xtile(j, t):
                return xt[:, j * sh + t * P: j * sh + (t + 1) * P]

            # ---------------- DRAM tables ----------------

            hag_in = [dr.tile([sh, D], H_DT, name=f"hag_in_{i}") for i in range(L)]
            h_full = [dr.tile([n_pad, D], H_DT, name=f"h_full_{i}", addr_space="Shared")
                      for i in range(L)]

            def allgather(in_t, out_t):
                if nocc:
                    nc.sync.dma_start(out=out_t[:sh], in_=in_t[:])
                else:
                    nc.gpsimd.collective_compute(
                        "AllGather", mybir.AluOpType.bypass,
                        replica_groups=[list(range(NC))],
                        ins=[in_t[:]], outs=[out_t[:]],
                    )

            # ---------------- stage 0: k,v shard tables + AG, then qT ----------------

            qT = []
            for j in range(DJ):
                qTj = sb.tile([P, sh], BF16, name=f"qT_{j}")
                n0 = 0
                while n0 < sh:
                    nn = min(512, sh - n0)
                    pq = ps.tile([P, 512], F32, name="pq", tag="pmm", bufs=kpmm)
                    for ji in range(DJ):
                        nc.tensor.matmul(
                            pq[:, :nn],
                            lhsT=wslice(WQ, ji)[:, j * P:(j + 1) * P],
                            rhs=xt[:, ji * sh + n0: ji * sh + n0 + nn],
                            start=(ji == 0), stop=(ji == DJ - 1),
                        )
                    nc.vector.tensor_scalar(
                        out=qTj[:, n0:n0 + nn], in0=pq[:, :nn],
                        scalar1=vp[:, 9 * D + j: 9 * D + j + 1], scalar2=None,
                        op0=mybir.AluOpType.add,
                    )
                    n0 += nn
                qT.append(qTj)

            # shard-resident activations
            h_cur = sb.tile([P, nt * D], H_DT)
            h_nxt = sb.tile([P, nt * D], H_DT)
            hT_cur = sb.tile([P, DJ * sh], D_DT)
            hT_nxt = sb.tile([P, DJ * sh], D_DT)

            def agg_pass(layer, h_prev, hT_prev, h_out, hT_out):
                """layer -1: transformer (h_prev/hT_prev unused); 0..L-1: SAGE."""
                li = layer + 1  # h table index this pass WRITES (0 for transformer)
                kh = khalf if layer >= 0 else int(os.environ.get("KHALFT", "3"))
                splits = []  # (c0, c1) chunk ranges per gather piece
                base = (S + kh - 1) // kh
                c0 = 0
                while c0 < S:
                    splits.append((c0, min(S, c0 + base)))
                    c0 += base
                # graded head for tile 0: a small leading split lets chunk-0
                # consumers start right after the pass-gating AllGather
                kgrad = int(os.environ.get("KGRAD", "0"))
                splits0 = splits
                if kgrad and layer >= 0:
                    splits0 = [(0, kgrad)]
                    c0 = kgrad
                    while c0 < S:
                        splits0.append((c0, min(S, c0 + base)))
                        c0 += base
                for t in range(nt):
                    if layer < 0:
                        kgt = gp.tile([P, DJ, ET], BF16, name="kgt", tag="kgt",
                                      bufs=int(os.environ.get("KKGT", "2")))
                        vg = gp.tile([P, S, D], V_DT, name="vg", tag="vg",
                                     bufs=int(os.environ.get("KVG", "3")))
                        pzB = None
                    else:
                        kgt = None
                        vg = gp.tile([P, S, D], H_DT, name="hg", tag="vg",
                                     bufs=int(os.environ.get("KVG", "3")))
                        # root + bias + gated residual, all on PE — independent
                        # of the gather; issue first so PE fills the wait.
                        pzB = ps.tile([P, D], F32, name="pzB", tag="pmm", bufs=kpmm)
                        for j in range(DJ):
                            nc.tensor.matmul(
                                pzB[:],
                                lhsT=hT_prev[:, j * sh + t * P: j * sh + (t + 1) * P],
                                rhs=wslice(WR[layer], j),
                                start=(j == 0), stop=False)
                        nc.tensor.matmul(
                            pzB[:], lhsT=ones1[:],
                            rhs=brow[0:1, (layer + 1) * D:(layer + 2) * D],
                            start=False, stop=False)
                        for j in range(DJ):
                            nc.tensor.matmul(
                                pzB[:],
                                lhsT=hT_prev[:, j * sh + t * P: j * sh + (t + 1) * P],
                                rhs=omaI[:, j * D:(j + 1) * D],
                                start=False, stop=(j == DJ - 1))
                    if layer < 0:
                        idx_tt = idx_sb[:, t * S * 8:(t + 1) * S * 8]
                        nc.gpsimd.dma_gather(
                            out_ap=kgt[:], in_ap=xfull_in[:], idxs_ap=idx_tt,
                            num_idxs=ET, num_idxs_reg=ET, elem_size=D,
                            transpose=True, single_packet=False)
                    src_tab = xfull_in if layer < 0 else h_full[layer]
                    for (ca, cb) in (splits0 if t == 0 else splits):
                        nn_i = (cb - ca) * P
                        idx_t = idx_sb[:, t * S * 8 + ca * 8: t * S * 8 + cb * 8]
                        nc.gpsimd.dma_gather(
                            out_ap=vg[:, ca:cb, :], in_ap=src_tab[:], idxs_ap=idx_t,
                            num_idxs=nn_i, num_idxs_reg=nn_i, elem_size=D,
                            single_packet=False)

                    pagg = ps.tile([P, D + 1], F32, name="pagg", tag="pagg", bufs=kpagg)
                    if layer >= 0:
                        # two transposed-scatter accumulators must sit in
                        # DIFFERENT psum banks (per-bank accumulation state)
                        pagg2 = ps.tile([P, P], F32, name="pagg2", tag="psc", bufs=kpsc)
                        sage_halves = [pagg[:, :P], pagg2[:]]
                    else:
                        sage_halves = None
                    kexp = int(os.environ.get("KEXP", "4"))  # chunks per exp call
                    # walrus rejects TensorScalarPtr on Pool; keep builds on DVE
                    kpool = int(os.environ.get("KPOOLB", "0"))
                    if layer < 0:
                        # batched scores: groups of kexp chunks share one psum
                        # bank and one exp() call (amortizes Act access lat.)
                        for c0 in range(0, S, kexp):
                            cn = min(kexp, S - c0)
                            pscw = ps.tile([P, kexp * P], F32, name="pscw",
                                           tag="psc", bufs=kpsc)
                            for ci in range(cn):
                                c = c0 + ci
                                for j in range(DJ):
                                    nc.tensor.matmul(
                                        pscw[:, ci * P:(ci + 1) * P],
                                        lhsT=kgt[:, j, c * P:(c + 1) * P],
                                        rhs=qT[j][:, t * P:(t + 1) * P],
                                        start=(j == 0), stop=(j == DJ - 1))
                            exps = smp.tile([P, kexp * P], BF16, name="exps")
                            nc.scalar.activation(exps[:, :cn * P], pscw[:, :cn * P],
                                                 mybir.ActivationFunctionType.Exp,
                                                 scale=scale)
                            for ci in range(cn):
                                c = c0 + ci
                                dcol = dstc[:, t * S + c: t * S + c + 1]
                                w_b = smp.tile([P, P], V_DT, name="w_b", tag="w_b")
                                if os.environ.get("KWBSTT", "1") == "1":
                                    nc.vector.scalar_tensor_tensor(
                                        out=w_b[:], in0=iota_b[:], scalar=dcol,
                                        in1=exps[:, ci * P:(ci + 1) * P],
                                        op0=mybir.AluOpType.is_equal,
                                        op1=mybir.AluOpType.mult)
                                else:
                                    # two simple all-bf16 ops hit the DVE 2x
                                    # path; the fused STT variant does not
                                    ind_t = smp.tile([P, P], BF16, name="ind_t",
                                                     tag="ind_t")
                                    nc.vector.tensor_scalar(
                                        out=ind_t[:], in0=iota_b[:], scalar1=dcol,
                                        scalar2=None, op0=mybir.AluOpType.is_equal)
                                    nc.vector.tensor_tensor(
                                        out=w_b[:], in0=ind_t[:],
                                        in1=exps[:, ci * P:(ci + 1) * P],
                                        op=mybir.AluOpType.mult)
                                nc.tensor.matmul(pagg[:, :D], lhsT=w_b[:],
                                                 rhs=vg[:, c, :],
                                                 start=(c == 0), stop=(c == S - 1))
                                nc.tensor.matmul(pagg[:, D:D + 1], lhsT=w_b[:],
                                                 rhs=ones_v[:],
                                                 start=False, stop=(c == S - 1))
                    if layer >= 0:
                        for c in range(S):
                            dcol = dstc[:, t * S + c: t * S + c + 1]
                            # scatter TRANSPOSED: paggT_j[d_j, dst]
                            # accumulates in two psum halves (distinct banks).
                            ind_b = smp.tile([P, P], H_DT, name="ind_b", tag="w_b")
                            nc.vector.tensor_scalar(
                                out=ind_b[:], in0=iota_b[:], scalar1=dcol,
                                scalar2=None, op0=mybir.AluOpType.is_equal)
                            for j in range(DJ):
                                nc.tensor.matmul(
                                    sage_halves[j],
                                    lhsT=vg[:, c, j * P:(j + 1) * P],
                                    rhs=ind_b[:],
                                    start=(c == 0), stop=(c == S - 1))

                    # ---- tile epilogue -> h_out tile [node, d] ----
                    if layer < 0:
                        smax = smp.tile([P, 1], F32, name="smax")
                        nc.vector.tensor_scalar(
                            out=smax[:], in0=pagg[:, D:D + 1], scalar1=1e-30,
                            scalar2=None, op0=mybir.AluOpType.max)
                        rs = smp.tile([P, 1], F32, name="rs")
                        nc.vector.reciprocal(rs[:], smax[:])
                        pskip = ps.tile([P, D], F32, name="pskip", tag="pmm", bufs=kpmm)
                        for ji in range(DJ):
                            nc.tensor.matmul(pskip[:], lhsT=xtile(ji, t),
                                             rhs=wslice(WS, ji),
                                             start=(ji == 0), stop=False)
                        nc.tensor.matmul(pskip[:], lhsT=ones1[:],
                                         rhs=brow[0:1, :D],
                                         start=False, stop=True)
                        aggx = smp.tile([P, D], BF16, name="aggx", tag="t1")
                        nc.scalar.copy(out=aggx[:], in_=pagg[:, :D])
                        pvagg = ps.tile([P, D], F32, name="pvagg", tag="pmm",
                                        bufs=kpmm)
                        for j in range(DJ):
                            ptrv = ps.tile([P, P], BF16, name="ptrv", tag="ptr",
                                           bufs=kptr)
                            nc.tensor.transpose(
                                out=ptrv[:], in_=aggx[:, j * P:(j + 1) * P],
                                identity=ident_b[:])
                            mTv = smp.tile([P, P], BF16, name="mTv", tag="mT")
                            nc.scalar.copy(out=mTv[:], in_=ptrv[:])
                            nc.tensor.matmul(pvagg[:], lhsT=mTv[:],
                                             rhs=wslice(WV, j),
                                             start=(j == 0), stop=(j == DJ - 1))
                        t1 = smp.tile([P, D], F32, name="t1", tag="t3")
                        nc.scalar.activation(t1[:], pvagg[:],
                                             mybir.ActivationFunctionType.Copy,
                                             scale=rs[:, :1])
                        t2 = smp.tile([P, D], F32, name="t2", tag="t2")
                        nc.vector.tensor_tensor(out=t2[:], in0=t1[:], in1=pskip[:],
                                                op=mybir.AluOpType.add)
                        nc.scalar.activation(h_out[:, t * D:(t + 1) * D], t2[:],
                                             mybir.ActivationFunctionType.Relu)
                    else:
                        # mean-term (gamma folded into Wl'): pzA = sumT^T @ Wl'
                        pzA = ps.tile([P, D], F32, name="pzA", tag="pmm", bufs=kpmm)
                        for j in range(DJ):
                            mT = smp.tile([P, P], D_DT, name="mT", tag="mT")
                            nc.scalar.copy(out=mT[:], in_=sage_halves[j])
                            nc.tensor.matmul(pzA[:], lhsT=mT[:],
                                             rhs=wslice(WL[layer], j),
                                             start=(j == 0), stop=(j == DJ - 1))
                        # gate(z) = al*z_affine + (1-al)h_prev: pzB already
                        # holds root+bias+residual; add mean term / deg.
                        t1 = smp.tile([P, D], F32, name="t1s", tag="t2")
                        nc.vector.tensor_scalar(
                            out=t1[:], in0=pzA[:], scalar1=invd[:, t:t + 1],
                            scalar2=None, op0=mybir.AluOpType.mult)
                        t2 = smp.tile([P, D], F32, name="t2s", tag="t3")
                        nc.vector.tensor_tensor(out=t2[:], in0=t1[:], in1=pzB[:],
                                                op=mybir.AluOpType.add)
                        if layer == L - 1:
                            houtf = smp.tile([P, D], F32, name="houtf", tag="t4")
                            nc.scalar.activation(houtf[:], t2[:],
                                                 mybir.ActivationFunctionType.Relu)
                            nc.sync.dma_start(out=out_dram[t * P:(t + 1) * P, :],
                                              in_=houtf[:])
                        else:
                            nc.scalar.activation(h_out[:, t * D:(t + 1) * D], t2[:],
                                                 mybir.ActivationFunctionType.Relu)

                    if layer < L - 1:
                        nc.sync.dma_start(out=hag_in[li][t * P:(t + 1) * P, :],
                                          in_=h_out[:, t * D:(t + 1) * D])
                        for j in range(DJ):
                            ptr2 = ps.tile([P, P], BF16, name="ptr2", tag="ptr", bufs=kptr)
                            nc.tensor.transpose(
                                out=ptr2[:],
                                in_=h_out[:, t * D + j * P: t * D + (j + 1) * P],
                                identity=ident_b[:])
                            nc.scalar.copy(
                                out=hT_out[:, j * sh + t * P: j * sh + (t + 1) * P],
                                in_=ptr2[:])

                if layer < L - 1:
                    allgather(hag_in[li], h_full[li])

            if stages <= 1:
                # dump k_full slice so the program has an output
                tmpo = smp.tile([P, D], F32, name="tmpo")
                for t in range(nt):
                    nc.vector.tensor_copy(out=tmpo[:], in_=xt[:, :D])
                    nc.sync.dma_start(out=out_dram[t * P:(t + 1) * P, :], in_=tmpo[:])
            else:
                agg_pass(-1, None, None, h_cur, hT_cur)
                bufs = [(h_cur, hT_cur), (h_nxt, hT_nxt)]
                for i in range(min(L, stages - 2)):
                    h_prev, hT_prev = bufs[i % 2]
                    h_out, hT_out = bufs[(i + 1) % 2]
                    agg_pass(i, h_prev, hT_prev, h_out, hT_out)
                if stages - 2 < L:
                    hsrc, _ = bufs[max(0, stages - 2) % 2]
                    for t in range(nt):
                        tmpo = smp.tile([P, D], F32, name="tmpo2")
                        nc.vector.tensor_copy(out=tmpo[:],
                                              in_=hsrc[:, t * D:(t + 1) * D])
                        nc.sync.dma_start(out=out_dram[t * P:(t + 1) * P, :],
                                          in_=tmpo[:])

    nc.compile()
    _nc_cache[key] = nc
    return nc


def _balance_perm(dst, n, n_pad):
    """Renumber nodes so each dst-tile of 128 has a near-equal edge count.
    Returns perm: old id -> new id (padding slots filled with virtual ids)."""
    import heapq

    deg = np.bincount(dst, minlength=n)
    nbins = n_pad // P
    counts = np.zeros(nbins, np.int64)
    perm = np.empty(n, np.int64)
    heap = [(0, g) for g in range(nbins)]
    heapq.heapify(heap)
    for node in np.argsort(-deg, kind="stable"):
        while True:
            load, g = heapq.heappop(heap)
            if counts[g] < P:
                break
        perm[node] = g * P + counts[g]
        counts[g] += 1
        heapq.heappush(heap, (load + int(deg[node]), g))
    return perm


def _host_prep(x, src, dst, Wq, bq, Wk, bk, Wv, bv, Ws, bs, Wl, bl, Wr,
               gamma, beta, alpha_res):
    n, d = x.shape
    n_pad = ((n + NC * P - 1) // (NC * P)) * (NC * P)
    sh = n_pad // NC
    nt = sh // P
    n_tiles = n_pad // P

    perm = _balance_perm(dst, n, n_pad)
    src = perm[src]
    dst = perm[dst]

    order = np.argsort(dst, kind="stable")
    src_s, dst_s = src[order], dst[order]
    tile_of = dst_s // P
    counts = np.bincount(tile_of, minlength=n_tiles)
    starts = np.concatenate([[0], np.cumsum(counts)])
    S = int(max(1, (counts.max() + P - 1) // P))
    ET = S * P

    deg = np.bincount(dst, minlength=n_pad).astype(np.float32)
    invdeg_full = 1.0 / np.maximum(deg, 1.0)

    al = 1.0 / (1.0 + np.exp(-alpha_res))
    oma = float(1.0 - al)
    bn_scale = 1.0 / np.sqrt(1.0 + BN_EPS)
    scale = 1.0 / np.sqrt(float(d))

    x_pad = np.zeros((n_pad, D), np.float32)
    x_pad[perm] = x
    xT = x_pad.T.copy()
    x_full_b = x_pad.astype(ml_dtypes.bfloat16)

    Gx = [al * bn_scale * gamma[i] for i in range(L)]
    Bx = [al * (bl[i] * bn_scale * gamma[i] + beta[i]) for i in range(L)]

    Wq_eff = (Wq.astype(np.float64) @ Wk.astype(np.float64).T).astype(np.float32)
    bq_eff = (bq.astype(np.float64) @ Wk.astype(np.float64).T).astype(np.float32)

    # gamma/bn/gate scale folded into the SAGE weights (per-output-column)
    weights = [Wq_eff, Wk, Wv, Ws,
               Wl[0] * Gx[0][None, :], Wr[0] * Gx[0][None, :],
               Wl[1] * Gx[1][None, :], Wr[1] * Gx[1][None, :],
               Wl[2] * Gx[2][None, :], Wr[2] * Gx[2][None, :]]
    wpack = np.zeros((P, 11 * DJ * D), D_NP)
    for w, W in enumerate(weights):
        for j in range(DJ):
            wpack[:, (w * DJ + j) * D:(w * DJ + j + 1) * D] = W[j * P:(j + 1) * P, :]
    for j in range(DJ):
        blk = np.zeros((P, D), np.float32)
        blk[np.arange(P), j * P + np.arange(P)] = oma
        wpack[:, (10 * DJ + j) * D:(10 * DJ + j + 1) * D] = blk
    vecs = [bk, bv, bs + bv, Gx[0], Bx[0], Gx[1], Bx[1], Gx[2], Bx[2]]
    vpack = np.empty((P, 9 * D + DJ), np.float32)
    for k, v in enumerate(vecs):
        vpack[:, k * D:(k + 1) * D] = np.tile(v[None, :], (P, 1))
    for j in range(DJ):
        vpack[:, 9 * D + j] = bq_eff[j * P:(j + 1) * P]

    in_maps = []
    for r in range(NC):
        idx_arr = np.zeros((P, nt * S * 8), np.int16)
        dst_arr = np.full((P, nt * S), 128.0, np.float32)
        for tloc in range(nt):
            g = r * nt + tloc
            e0, e1 = starts[g], starts[g + 1]
            cnt = e1 - e0
            srcs = np.zeros(ET, np.int64)
            srcs[:cnt] = src_s[e0:e1]
            dl = np.full(ET, 128, np.int64)
            dl[:cnt] = dst_s[e0:e1] - g * P
            idx_arr[:, tloc * S * 8:(tloc + 1) * S * 8] = _wrap_idx(srcs)
            dst_arr[:, tloc * S:(tloc + 1) * S] = dl.reshape(S, P).T
        invdeg_r = invdeg_full[r * sh:(r + 1) * sh].reshape(nt, P).T.copy()

        xt_r = np.empty((P, DJ * sh), D_NP)
        for j in range(DJ):
            xt_r[:, j * sh:(j + 1) * sh] = xT[j * P:(j + 1) * P, r * sh:(r + 1) * sh]

        in_maps.append({
            "xt_in": xt_r,
            "xfull_in": x_full_b,
            "wpack_in": wpack,
            "vpack_in": vpack,
            "idx_in": idx_arr,
            "dst_in": dst_arr,
            "invdeg_in": np.ascontiguousarray(invdeg_r),
        })
    return in_maps, (n_pad, sh, nt, S, scale, oma), perm


def kernel(**inputs):
    x = np.asarray(inputs["x"], np.float32)
    edge_index = np.asarray(inputs["edge_index"])
    args = dict(
        Wq=np.asarray(inputs["Wq"], np.float32), bq=np.asarray(inputs["bq"], np.float32),
        Wk=np.asarray(inputs["Wk"], np.float32), bk=np.asarray(inputs["bk"], np.float32),
        Wv=np.asarray(inputs["Wv"], np.float32), bv=np.asarray(inputs["bv"], np.float32),
        Ws=np.asarray(inputs["Ws"], np.float32), bs=np.asarray(inputs["bs"], np.float32),
        Wl=np.asarray(inputs["Wl"], np.float32), bl=np.asarray(inputs["bl"], np.float32),
        Wr=np.asarray(inputs["Wr"], np.float32),
        gamma=np.asarray(inputs["gamma"], np.float32),
        beta=np.asarray(inputs["beta"], np.float32),
        alpha_res=float(np.asarray(inputs["alpha_res"])),
    )
    src = edge_index[0].astype(np.int64)
    dst = edge_index[1].astype(np.int64)

    in_maps, (n_pad, sh, nt, S, scale, oma), perm = _host_prep(x, src, dst, **args)
    t0 = time.time()
    nc = build_nc(n_pad, sh, nt, S, scale, oma)
    print(f"[kernel] build+compile {time.time()-t0:.1f}s", flush=True)
    t0 = time.time()
    res = run_bass_kernel_spmd(nc, in_maps, core_ids=list(range(NC)))
    print(f"[kernel] run {time.time()-t0:.1f}s", flush=True)
    out = np.concatenate([res.results[r]["out"] for r in range(NC)], axis=0)
    return out[perm]



# revision 93
# speedup vs baseline: 1.1067x; 1.0082x over previous
"""Trainium2 Bass kernel for nn_MixGNN (TransformerConv + 3x SAGEConv + BN + gated residual).

Strategy (8 NeuronCores, dst-node sharding):
  - Pad N 10000 -> 10240; core r owns 1280 dst nodes = 10 tiles of 128.
  - Host prep: degree-balanced node renumbering (each dst-tile gets ~E/80
    edges, shrinking the padded chunk count S), edges sorted by dst and
    bucketed per tile, wrapped int16 gather indices, per-chunk local-dst
    columns, 1/deg, bf16-packed weights with BN gamma/gate folded into
    Wl/Wr columns and an extra oma*I block for the PE-side residual.
  - Device per layer: dense matmuls on PE with bf16 operands (1 cycle/row);
    per-edge work via dma_gather of source rows (512B descriptors) +
    indicator matmuls (Ind[e,n] = (dst_e==n) built by DVE is_equal).
    Transformer: scores KgT.T @ qT from a transposed bf16 gather, batched
    exp() over 4 score chunks per Act call, w_b = Ind*exp via one fused DVE
    op, agg + exp-sum accumulated in one PSUM group; epilogue adds x@Ws with
    the bias via a rank-1 ones-row matmul. SAGE: TRANSPOSED scatter
    (paggT_j[d_j,dst] in two distinct PSUM banks), pzA = sumT^T @ Wl' with
    1/deg applied after (commutes), and pzB = h@Wr' + bias + oma*h computed
    entirely on PE (bias via ones-row, residual via oma*I against hT) and
    issued before the gathers so PE fills the wait; epilogue is 2 DVE ops +
    relu, with non-final h kept bf16 so relu output feeds the AllGather
    staging and the hT transposes directly.
  - Halo exchange: AllGather of each core's h shard (bf16) into a full table
    in shared DRAM before every aggregation.
Output: fp32 [10000, 256].
"""
import os
import sys
import time

import numpy as np

for _p in ("/opt/trn_rl_repo",):
    if _p not in sys.path:
        sys.path.insert(0, _p)

import ml_dtypes  # noqa: E402
import concourse.bacc as bacc  # noqa: E402
import concourse.mybir as mybir  # noqa: E402
import concourse.tile as tile  # noqa: E402
from concourse.bass_utils import run_bass_kernel_spmd  # noqa: E402

P = 128
D = 256
DJ = D // P           # 2 d-chunks of 128
NC = 8                # cores
L = 3                 # SAGE layers
BN_EPS = 1e-5

# dtype knobs for gathered tables (accuracy vs bandwidth)
V_BF16 = True         # v table + attention agg in bf16
H_BF16 = True         # h tables + SAGE agg in bf16

F32 = mybir.dt.float32
F32R = (mybir.dt.float32 if os.environ.get("KNOF32R") == "1"
        else mybir.dt.float32r)  # f32 bits, PE full-rate mode at >=256 free
BF16 = mybir.dt.bfloat16
I16 = mybir.dt.int16
# dense-path operand dtype: bf16 halves DMA + SBUF for x/W/hT/mT tables
DENSE_BF16 = os.environ.get("KDENSE", "bf16") == "bf16"
D_DT = BF16 if DENSE_BF16 else F32R
D_NP = ml_dtypes.bfloat16 if DENSE_BF16 else np.float32
V_DT = BF16 if V_BF16 else F32
H_DT = BF16 if H_BF16 else F32

_nc_cache = {}


def _wrap_idx(a):
    """[S*128] int array -> [128, S*8] int16 wrapped gather-index layout."""
    w16 = a.reshape(-1, 16).T.astype(np.int16)   # [16, S*8]
    return np.tile(w16, (8, 1))                  # replicate to 8 Q7 stripes


def build_nc(n_pad, sh, nt, S, scale, oma):
    stages = int(os.environ.get("KSTAGES", "5"))
    nocc = os.environ.get("KNOCC") == "1"
    ksm = int(os.environ.get("KSM", "12"))
    kgp = int(os.environ.get("KGP", "2"))
    kpsc = int(os.environ.get("KPSC", "3"))
    kptr = int(os.environ.get("KPTR", "2"))
    kpagg = int(os.environ.get("KPAGG", "1"))
    kpmm = int(os.environ.get("KPMM", "2"))
    khalf = int(os.environ.get("KHALF", "4"))  # gather splits per tile
    kabl = os.environ.get("KABL", "")
    key = (n_pad, sh, nt, S, round(scale, 9), round(oma, 9), V_BF16, H_BF16, stages,
           nocc, ksm, kgp, kpsc, kptr, kpagg, kpmm, khalf, kabl,
           os.environ.get("KHALFT"),
           os.environ.get("KKGT"), os.environ.get("KVG"),
           os.environ.get("KEXP"), os.environ.get("KPOOLB"),
           os.environ.get("KWBSTT"), os.environ.get("KGRAD"))
    if key in _nc_cache:
        return _nc_cache[key]

    ET = S * P  # padded edges per tile
    ndev = 1 if nocc else NC
    nc = bacc.Bacc("TRN2", target_bir_lowering=False, debug=False, num_devices=ndev)

    xt_in = nc.dram_tensor("xt_in", [P, DJ * sh], D_DT, kind="ExternalInput")
    wpack_in = nc.dram_tensor("wpack_in", [P, 11 * DJ * D], D_DT, kind="ExternalInput")
    vpack_in = nc.dram_tensor("vpack_in", [P, 9 * D + DJ], F32, kind="ExternalInput")
    idx_in = nc.dram_tensor("idx_in", [P, nt * S * 8], I16, kind="ExternalInput")
    dst_in = nc.dram_tensor("dst_in", [P, nt * S], F32, kind="ExternalInput")
    invdeg_in = nc.dram_tensor("invdeg_in", [P, nt], F32, kind="ExternalInput")
    xfull_in = nc.dram_tensor("xfull_in", [n_pad, D], D_DT, kind="ExternalInput")
    out_dram = nc.dram_tensor("out", [sh, D], F32, kind="ExternalOutput")

    WQ, WK, WV, WS = 0, 1, 2, 3
    WL = [4, 6, 8]
    WR = [5, 7, 9]
    VBK, VBV, VBS = 0, 1, 2

    with tile.TileContext(nc) as tc:
        with (
            tc.tile_pool(name="cst", bufs=1) as cst,
            tc.tile_pool(name="sb", bufs=1) as sb,
            tc.tile_pool(name="g", bufs=kgp) as gp,
            tc.tile_pool(name="sm", bufs=ksm) as smp,
            tc.tile_pool(name="ps", bufs=2, space="PSUM") as ps,
            tc.tile_pool(name="dr", bufs=1, space="DRAM") as dr,
        ):
            # ---------------- constants / inputs to SBUF ----------------
            wp = cst.tile([P, 11 * DJ * D], D_DT)
            nc.sync.dma_start(out=wp[:], in_=wpack_in[:])
            vp = cst.tile([P, 9 * D + DJ], F32)
            nc.sync.dma_start(out=vp[:], in_=vpack_in[:])
            xt = cst.tile([P, DJ * sh], D_DT)
            for _xi in range(4):
                _c0 = _xi * (DJ * sh // 4)
                _c1 = (_xi + 1) * (DJ * sh // 4)
                nc.sync.dma_start(out=xt[:, _c0:_c1], in_=xt_in[:, _c0:_c1])
            dstc = cst.tile([P, nt * S], F32)
            nc.sync.dma_start(out=dstc[:], in_=dst_in[:])
            invd = cst.tile([P, nt], F32)
            nc.sync.dma_start(out=invd[:], in_=invdeg_in[:])
            idx_sb = cst.tile([P, nt * S * 8], I16)
            nc.sync.dma_start(out=idx_sb[:], in_=idx_in[:])

            iota_i = cst.tile([P, P], mybir.dt.int32)
            nc.gpsimd.iota(iota_i[:], pattern=[[1, P]], base=0, channel_multiplier=0)
            ones_v = cst.tile([P, 1], V_DT)
            nc.vector.memset(ones_v[:], 1.0)
            # identity for PE transposes: (iota_row == partition_idx)
            iota_part = cst.tile([P, 1], mybir.dt.int32)
            nc.gpsimd.iota(iota_part[:], pattern=[[1, 1]], base=0, channel_multiplier=1)
            iota_part_f = cst.tile([P, 1], F32)
            nc.vector.tensor_copy(out=iota_part_f[:], in_=iota_part[:])
            iota_b = cst.tile([P, P], BF16)
            nc.vector.tensor_copy(out=iota_b[:], in_=iota_i[:])
            ident_b = cst.tile([P, P], BF16)
            nc.vector.tensor_scalar(
                out=ident_b[:], in0=iota_b[:], scalar1=iota_part_f[:, :1],
                scalar2=None, op0=mybir.AluOpType.is_equal,
            )
            # rank-1 bias rows + oma*I blocks (bias/residual folded into PE)
            ones1 = cst.tile([1, P], BF16)
            nc.vector.memset(ones1[:], 1.0)
            brow = cst.tile([1, (L + 1) * D], BF16)
            nc.vector.tensor_copy(out=brow[0:1, :D], in_=vp[0:1, 2 * D:3 * D])
            for _i in range(L):
                nc.vector.tensor_copy(
                    out=brow[0:1, (_i + 1) * D:(_i + 2) * D],
                    in_=vp[0:1, (4 + 2 * _i) * D:(5 + 2 * _i) * D])
            omaI = wp[:, 10 * DJ * D:11 * DJ * D]  # host-packed oma*I blocks

            def wslice(w, j):
                return wp[:, (w * DJ + j) * D:(w * DJ + j + 1) * D]

            def vslice(k):
                return vp[:, k * D:(k + 1) * D]

            def xtile(j, t):
                return xt[:, j * sh + t * P: j * sh + (t + 1) * P]

            # ---------------- DRAM tables ----------------

            hag_in = [dr.tile([sh, D], H_DT, name=f"hag_in_{i}") for i in range(L)]
            h_full = [dr.tile([n_pad, D], H_DT, name=f"h_full_{i}", addr_space="Shared")
                      for i in range(L)]

            def allgather(in_t, out_t):
                if nocc:
                    nc.sync.dma_start(out=out_t[:sh], in_=in_t[:])
                else:
                    nc.gpsimd.collective_compute(
                        "AllGather", mybir.AluOpType.bypass,
                        replica_groups=[list(range(NC))],
                        ins=[in_t[:]], outs=[out_t[:]],
                    )

            # ---------------- stage 0: k,v shard tables + AG, then qT ----------------

            qT = []
            for j in range(DJ):
                qTj = sb.tile([P, sh], BF16, name=f"qT_{j}")
                n0 = 0
                while n0 < sh:
                    nn = min(512, sh - n0)
                    pq = ps.tile([P, 512], F32, name="pq", tag="pmm", bufs=kpmm)
                    for ji in range(DJ):
                        nc.tensor.matmul(
                            pq[:, :nn],
                            lhsT=wslice(WQ, ji)[:, j * P:(j + 1) * P],
                            rhs=xt[:, ji * sh + n0: ji * sh + n0 + nn],
                            start=(ji == 0), stop=(ji == DJ - 1),
                        )
                    nc.vector.tensor_scalar(
                        out=qTj[:, n0:n0 + nn], in0=pq[:, :nn],
                        scalar1=vp[:, 9 * D + j: 9 * D + j + 1], scalar2=None,
                        op0=mybir.AluOpType.add,
                    )
                    n0 += nn
                qT.append(qTj)

            # shard-resident activations
            h_cur = sb.tile([P, nt * D], H_DT)
            h_nxt = sb.tile([P, nt * D], H_DT)
            hT_cur = sb.tile([P, DJ * sh], D_DT)
            hT_nxt = sb.tile([P, DJ * sh], D_DT)

            def agg_pass(layer, h_prev, hT_prev, h_out, hT_out):
                """layer -1: transformer (h_prev/hT_prev unused); 0..L-1: SAGE."""
                li = layer + 1  # h table index this pass WRITES (0 for transformer)
                kh = khalf if layer >= 0 else int(os.environ.get("KHALFT", "4"))
                splits = []  # (c0, c1) chunk ranges per gather piece
                base = (S + kh - 1) // kh
                c0 = 0
                while c0 < S:
                    splits.append((c0, min(S, c0 + base)))
                    c0 += base
                # graded head for tile 0: a small leading split lets chunk-0
                # consumers start right after the pass-gating AllGather
                kgrad = int(os.environ.get("KGRAD", "0"))
                splits0 = splits
                if kgrad and layer >= 0:
                    splits0 = [(0, kgrad)]
                    c0 = kgrad
                    while c0 < S:
                        splits0.append((c0, min(S, c0 + base)))
                        c0 += base
                for t in range(nt):
                    if layer < 0:
                        kgt = gp.tile([P, DJ, ET], BF16, name="kgt", tag="kgt",
                                      bufs=int(os.environ.get("KKGT", "2")))
                        vg = gp.tile([P, S, D], V_DT, name="vg", tag="vg",
                                     bufs=int(os.environ.get("KVG", "3")))
                        pzB = None
                    else:
                        kgt = None
                        vg = gp.tile([P, S, D], H_DT, name="hg", tag="vg",
                                     bufs=int(os.environ.get("KVG", "3")))
                        # root + bias + gated residual, all on PE — independent
                        # of the gather; issue first so PE fills the wait.
                        pzB = ps.tile([P, D], F32, name="pzB", tag="pmm", bufs=kpmm)
                        for j in range(DJ):
                            nc.tensor.matmul(
                                pzB[:],
                                lhsT=hT_prev[:, j * sh + t * P: j * sh + (t + 1) * P],
                                rhs=wslice(WR[layer], j),
                                start=(j == 0), stop=False)
                        nc.tensor.matmul(
                            pzB[:], lhsT=ones1[:],
                            rhs=brow[0:1, (layer + 1) * D:(layer + 2) * D],
                            start=False, stop=False)
                        for j in range(DJ):
                            nc.tensor.matmul(
                                pzB[:],
                                lhsT=hT_prev[:, j * sh + t * P: j * sh + (t + 1) * P],
                                rhs=omaI[:, j * D:(j + 1) * D],
                                start=False, stop=(j == DJ - 1))
                    if layer < 0:
                        idx_tt = idx_sb[:, t * S * 8:(t + 1) * S * 8]
                        nc.gpsimd.dma_gather(
                            out_ap=kgt[:], in_ap=xfull_in[:], idxs_ap=idx_tt,
                            num_idxs=ET, num_idxs_reg=ET, elem_size=D,
                            transpose=True, single_packet=False)
                    src_tab = xfull_in if layer < 0 else h_full[layer]
                    for (ca, cb) in (splits0 if t == 0 else splits):
                        nn_i = (cb - ca) * P
                        idx_t = idx_sb[:, t * S * 8 + ca * 8: t * S * 8 + cb * 8]
                        nc.gpsimd.dma_gather(
                            out_ap=vg[:, ca:cb, :], in_ap=src_tab[:], idxs_ap=idx_t,
                            num_idxs=nn_i, num_idxs_reg=nn_i, elem_size=D,
                            single_packet=False)

                    pagg = ps.tile([P, D + 1], F32, name="pagg", tag="pagg", bufs=kpagg)
                    if layer >= 0:
                        # two transposed-scatter accumulators must sit in
                        # DIFFERENT psum banks (per-bank accumulation state)
                        pagg2 = ps.tile([P, P], F32, name="pagg2", tag="psc", bufs=kpsc)
                        sage_halves = [pagg[:, :P], pagg2[:]]
                    else:
                        sage_halves = None
                    kexp = int(os.environ.get("KEXP", "4"))  # chunks per exp call
                    # walrus rejects TensorScalarPtr on Pool; keep builds on DVE
                    kpool = int(os.environ.get("KPOOLB", "0"))
                    if layer < 0:
                        # batched scores: groups of kexp chunks share one psum
                        # bank and one exp() call (amortizes Act access lat.)
                        for c0 in range(0, S, kexp):
                            cn = min(kexp, S - c0)
                            pscw = ps.tile([P, kexp * P], F32, name="pscw",
                                           tag="psc", bufs=kpsc)
                            for ci in range(cn):
                                c = c0 + ci
                                for j in range(DJ):
                                    nc.tensor.matmul(
                                        pscw[:, ci * P:(ci + 1) * P],
                                        lhsT=kgt[:, j, c * P:(c + 1) * P],
                                        rhs=qT[j][:, t * P:(t + 1) * P],
                                        start=(j == 0), stop=(j == DJ - 1))
                            exps = smp.tile([P, kexp * P], BF16, name="exps")
                            nc.scalar.activation(exps[:, :cn * P], pscw[:, :cn * P],
                                                 mybir.ActivationFunctionType.Exp,
                                                 scale=scale)
                            for ci in range(cn):
                                c = c0 + ci
                                dcol = dstc[:, t * S + c: t * S + c + 1]
                                w_b = smp.tile([P, P], V_DT, name="w_b", tag="w_b")
                                if os.environ.get("KWBSTT", "1") == "1":
                                    nc.vector.scalar_tensor_tensor(
                                        out=w_b[:], in0=iota_b[:], scalar=dcol,
                                        in1=exps[:, ci * P:(ci + 1) * P],
                                        op0=mybir.AluOpType.is_equal,
                                        op1=mybir.AluOpType.mult)
                                else:
                                    # two simple all-bf16 ops hit the DVE 2x
                                    # path; the fused STT variant does not
                                    ind_t = smp.tile([P, P], BF16, name="ind_t",
                                                     tag="ind_t")
                                    nc.vector.tensor_scalar(
                                        out=ind_t[:], in0=iota_b[:], scalar1=dcol,
                                        scalar2=None, op0=mybir.AluOpType.is_equal)
                                    nc.vector.tensor_tensor(
                                        out=w_b[:], in0=ind_t[:],
                                        in1=exps[:, ci * P:(ci + 1) * P],
                                        op=mybir.AluOpType.mult)
                                nc.tensor.matmul(pagg[:, :D], lhsT=w_b[:],
                                                 rhs=vg[:, c, :],
                                                 start=(c == 0), stop=(c == S - 1))
                                nc.tensor.matmul(pagg[:, D:D + 1], lhsT=w_b[:],
                                                 rhs=ones_v[:],
                                                 start=False, stop=(c == S - 1))
                    if layer >= 0:
                        for c in range(S):
                            dcol = dstc[:, t * S + c: t * S + c + 1]
                            # scatter TRANSPOSED: paggT_j[d_j, dst]
                            # accumulates in two psum halves (distinct banks).
                            ind_b = smp.tile([P, P], H_DT, name="ind_b", tag="w_b")
                            nc.vector.tensor_scalar(
                                out=ind_b[:], in0=iota_b[:], scalar1=dcol,
                                scalar2=None, op0=mybir.AluOpType.is_equal)
                            for j in range(DJ):
                                nc.tensor.matmul(
                                    sage_halves[j],
                                    lhsT=vg[:, c, j * P:(j + 1) * P],
                                    rhs=ind_b[:],
                                    start=(c == 0), stop=(c == S - 1))

                    # ---- tile epilogue -> h_out tile [node, d] ----
                    if layer < 0:
                        smax = smp.tile([P, 1], F32, name="smax")
                        nc.vector.tensor_scalar(
                            out=smax[:], in0=pagg[:, D:D + 1], scalar1=1e-30,
                            scalar2=None, op0=mybir.AluOpType.max)
                        rs = smp.tile([P, 1], F32, name="rs")
                        nc.vector.reciprocal(rs[:], smax[:])
                        pskip = ps.tile([P, D], F32, name="pskip", tag="pmm", bufs=kpmm)
                        for ji in range(DJ):
                            nc.tensor.matmul(pskip[:], lhsT=xtile(ji, t),
                                             rhs=wslice(WS, ji),
                                             start=(ji == 0), stop=False)
                        nc.tensor.matmul(pskip[:], lhsT=ones1[:],
                                         rhs=brow[0:1, :D],
                                         start=False, stop=True)
                        aggx = smp.tile([P, D], BF16, name="aggx", tag="t1")
                        nc.scalar.copy(out=aggx[:], in_=pagg[:, :D])
                        pvagg = ps.tile([P, D], F32, name="pvagg", tag="pmm",
                                        bufs=kpmm)
                        for j in range(DJ):
                            ptrv = ps.tile([P, P], BF16, name="ptrv", tag="ptr",
                                           bufs=kptr)
                            nc.tensor.transpose(
                                out=ptrv[:], in_=aggx[:, j * P:(j + 1) * P],
                                identity=ident_b[:])
                            mTv = smp.tile([P, P], BF16, name="mTv", tag="mT")
                            nc.scalar.copy(out=mTv[:], in_=ptrv[:])
                            nc.tensor.matmul(pvagg[:], lhsT=mTv[:],
                                             rhs=wslice(WV, j),
                                             start=(j == 0), stop=(j == DJ - 1))
                        t1 = smp.tile([P, D], F32, name="t1", tag="t3")
                        nc.scalar.activation(t1[:], pvagg[:],
                                             mybir.ActivationFunctionType.Copy,
                                             scale=rs[:, :1])
                        t2 = smp.tile([P, D], F32, name="t2", tag="t2")
                        nc.vector.tensor_tensor(out=t2[:], in0=t1[:], in1=pskip[:],
                                                op=mybir.AluOpType.add)
                        nc.scalar.activation(h_out[:, t * D:(t + 1) * D], t2[:],
                                             mybir.ActivationFunctionType.Relu)
                    else:
                        # mean-term (gamma folded into Wl'): pzA = sumT^T @ Wl'
                        pzA = ps.tile([P, D], F32, name="pzA", tag="pmm", bufs=kpmm)
                        for j in range(DJ):
                            mT = smp.tile([P, P], D_DT, name="mT", tag="mT")
                            nc.scalar.copy(out=mT[:], in_=sage_halves[j])
                            nc.tensor.matmul(pzA[:], lhsT=mT[:],
                                             rhs=wslice(WL[layer], j),
                                             start=(j == 0), stop=(j == DJ - 1))
                        # gate(z) = al*z_affine + (1-al)h_prev: pzB already
                        # holds root+bias+residual; add mean term / deg.
                        t1 = smp.tile([P, D], F32, name="t1s", tag="t2")
                        nc.vector.tensor_scalar(
                            out=t1[:], in0=pzA[:], scalar1=invd[:, t:t + 1],
                            scalar2=None, op0=mybir.AluOpType.mult)
                        t2 = smp.tile([P, D], F32, name="t2s", tag="t3")
                        nc.vector.tensor_tensor(out=t2[:], in0=t1[:], in1=pzB[:],
                                                op=mybir.AluOpType.add)
                        if layer == L - 1:
                            houtf = smp.tile([P, D], F32, name="houtf", tag="t4")
                            nc.scalar.activation(houtf[:], t2[:],
                                                 mybir.ActivationFunctionType.Relu)
                            nc.sync.dma_start(out=out_dram[t * P:(t + 1) * P, :],
                                              in_=houtf[:])
                        else:
                            nc.scalar.activation(h_out[:, t * D:(t + 1) * D], t2[:],
                                                 mybir.ActivationFunctionType.Relu)

                    if layer < L - 1:
                        nc.sync.dma_start(out=hag_in[li][t * P:(t + 1) * P, :],
                                          in_=h_out[:, t * D:(t + 1) * D])
                        for j in range(DJ):
                            ptr2 = ps.tile([P, P], BF16, name="ptr2", tag="ptr", bufs=kptr)
                            nc.tensor.transpose(
                                out=ptr2[:],
                                in_=h_out[:, t * D + j * P: t * D + (j + 1) * P],
                                identity=ident_b[:])
                            nc.scalar.copy(
                                out=hT_out[:, j * sh + t * P: j * sh + (t + 1) * P],
                                in_=ptr2[:])

                if layer < L - 1:
                    allgather(hag_in[li], h_full[li])

            if stages <= 1:
                # dump k_full slice so the program has an output
                tmpo = smp.tile([P, D], F32, name="tmpo")
                for t in range(nt):
                    nc.vector.tensor_copy(out=tmpo[:], in_=xt[:, :D])
                    nc.sync.dma_start(out=out_dram[t * P:(t + 1) * P, :], in_=tmpo[:])
            else:
                agg_pass(-1, None, None, h_cur, hT_cur)
                bufs = [(h_cur, hT_cur), (h_nxt, hT_nxt)]
                for i in range(min(L, stages - 2)):
                    h_prev, hT_prev = bufs[i % 2]
                    h_out, hT_out = bufs[(i + 1) % 2]
                    agg_pass(i, h_prev, hT_prev, h_out, hT_out)
                if stages - 2 < L:
                    hsrc, _ = bufs[max(0, stages - 2) % 2]
                    for t in range(nt):
                        tmpo = smp.tile([P, D], F32, name="tmpo2")
                        nc.vector.tensor_copy(out=tmpo[:],
                                              in_=hsrc[:, t * D:(t + 1) * D])
                        nc.sync.dma_start(out=out_dram[t * P:(t + 1) * P, :],
                                          in_=tmpo[:])

    nc.compile()
    _nc_cache[key] = nc
    return nc


def _balance_perm(dst, n, n_pad):
    """Renumber nodes so each dst-tile of 128 has a near-equal edge count.
    Returns perm: old id -> new id (padding slots filled with virtual ids)."""
    import heapq

    deg = np.bincount(dst, minlength=n)
    nbins = n_pad // P
    counts = np.zeros(nbins, np.int64)
    perm = np.empty(n, np.int64)
    heap = [(0, g) for g in range(nbins)]
    heapq.heapify(heap)
    for node in np.argsort(-deg, kind="stable"):
        while True:
            load, g = heapq.heappop(heap)
            if counts[g] < P:
                break
        perm[node] = g * P + counts[g]
        counts[g] += 1
        heapq.heappush(heap, (load + int(deg[node]), g))
    return perm


def _host_prep(x, src, dst, Wq, bq, Wk, bk, Wv, bv, Ws, bs, Wl, bl, Wr,
               gamma, beta, alpha_res):
    n, d = x.shape
    n_pad = ((n + NC * P - 1) // (NC * P)) * (NC * P)
    sh = n_pad // NC
    nt = sh // P
    n_tiles = n_pad // P

    perm = _balance_perm(dst, n, n_pad)
    src = perm[src]
    dst = perm[dst]

    order = np.argsort(dst, kind="stable")
    src_s, dst_s = src[order], dst[order]
    tile_of = dst_s // P
    counts = np.bincount(tile_of, minlength=n_tiles)
    starts = np.concatenate([[0], np.cumsum(counts)])
    S = int(max(1, (counts.max() + P - 1) // P))
    ET = S * P

    deg = np.bincount(dst, minlength=n_pad).astype(np.float32)
    invdeg_full = 1.0 / np.maximum(deg, 1.0)

    al = 1.0 / (1.0 + np.exp(-alpha_res))
    oma = float(1.0 - al)
    bn_scale = 1.0 / np.sqrt(1.0 + BN_EPS)
    scale = 1.0 / np.sqrt(float(d))

    x_pad = np.zeros((n_pad, D), np.float32)
    x_pad[perm] = x
    xT = x_pad.T.copy()
    x_full_b = x_pad.astype(ml_dtypes.bfloat16)

    Gx = [al * bn_scale * gamma[i] for i in range(L)]
    Bx = [al * (bl[i] * bn_scale * gamma[i] + beta[i]) for i in range(L)]

    Wq_eff = (Wq.astype(np.float64) @ Wk.astype(np.float64).T).astype(np.float32)
    bq_eff = (bq.astype(np.float64) @ Wk.astype(np.float64).T).astype(np.float32)

    # gamma/bn/gate scale folded into the SAGE weights (per-output-column)
    weights = [Wq_eff, Wk, Wv, Ws,
               Wl[0] * Gx[0][None, :], Wr[0] * Gx[0][None, :],
               Wl[1] * Gx[1][None, :], Wr[1] * Gx[1][None, :],
               Wl[2] * Gx[2][None, :], Wr[2] * Gx[2][None, :]]
    wpack = np.zeros((P, 11 * DJ * D), D_NP)
    for w, W in enumerate(weights):
        for j in range(DJ):
            wpack[:, (w * DJ + j) * D:(w * DJ + j + 1) * D] = W[j * P:(j + 1) * P, :]
    for j in range(DJ):
        blk = np.zeros((P, D), np.float32)
        blk[np.arange(P), j * P + np.arange(P)] = oma
        wpack[:, (10 * DJ + j) * D:(10 * DJ + j + 1) * D] = blk
    vecs = [bk, bv, bs + bv, Gx[0], Bx[0], Gx[1], Bx[1], Gx[2], Bx[2]]
    vpack = np.empty((P, 9 * D + DJ), np.float32)
    for k, v in enumerate(vecs):
        vpack[:, k * D:(k + 1) * D] = np.tile(v[None, :], (P, 1))
    for j in range(DJ):
        vpack[:, 9 * D + j] = bq_eff[j * P:(j + 1) * P]

    in_maps = []
    for r in range(NC):
        idx_arr = np.zeros((P, nt * S * 8), np.int16)
        dst_arr = np.full((P, nt * S), 128.0, np.float32)
        for tloc in range(nt):
            g = r * nt + tloc
            e0, e1 = starts[g], starts[g + 1]
            cnt = e1 - e0
            srcs = np.zeros(ET, np.int64)
            srcs[:cnt] = src_s[e0:e1]
            dl = np.full(ET, 128, np.int64)
            dl[:cnt] = dst_s[e0:e1] - g * P
            idx_arr[:, tloc * S * 8:(tloc + 1) * S * 8] = _wrap_idx(srcs)
            dst_arr[:, tloc * S:(tloc + 1) * S] = dl.reshape(S, P).T
        invdeg_r = invdeg_full[r * sh:(r + 1) * sh].reshape(nt, P).T.copy()

        xt_r = np.empty((P, DJ * sh), D_NP)
        for j in range(DJ):
            xt_r[:, j * sh:(j + 1) * sh] = xT[j * P:(j + 1) * P, r * sh:(r + 1) * sh]

        in_maps.append({
            "xt_in": xt_r,
            "xfull_in": x_full_b,
            "wpack_in": wpack,
            "vpack_in": vpack,
            "idx_in": idx_arr,
            "dst_in": dst_arr,
            "invdeg_in": np.ascontiguousarray(invdeg_r),
        })
    return in_maps, (n_pad, sh, nt, S, scale, oma), perm


def kernel(**inputs):
    x = np.asarray(inputs["x"], np.float32)
    edge_index = np.asarray(inputs["edge_index"])
    args = dict(
        Wq=np.asarray(inputs["Wq"], np.float32), bq=np.asarray(inputs["bq"], np.float32),
        Wk=np.asarray(inputs["Wk"], np.float32), bk=np.asarray(inputs["bk"], np.float32),
        Wv=np.asarray(inputs["Wv"], np.float32), bv=np.asarray(inputs["bv"], np.float32),
        Ws=np.asarray(inputs["Ws"], np.float32), bs=np.asarray(inputs["bs"], np.float32),
        Wl=np.asarray(inputs["Wl"], np.float32), bl=np.asarray(inputs["bl"], np.float32),
        Wr=np.asarray(inputs["Wr"], np.float32),
        gamma=np.asarray(inputs["gamma"], np.float32),
        beta=np.asarray(inputs["beta"], np.float32),
        alpha_res=float(np.asarray(inputs["alpha_res"])),
    )
    src = edge_index[0].astype(np.int64)
    dst = edge_index[1].astype(np.int64)

    in_maps, (n_pad, sh, nt, S, scale, oma), perm = _host_prep(x, src, dst, **args)
    t0 = time.time()
    nc = build_nc(n_pad, sh, nt, S, scale, oma)
    print(f"[kernel] build+compile {time.time()-t0:.1f}s", flush=True)
    t0 = time.time()
    res = run_bass_kernel_spmd(nc, in_maps, core_ids=list(range(NC)))
    print(f"[kernel] run {time.time()-t0:.1f}s", flush=True)
    out = np.concatenate([res.results[r]["out"] for r in range(NC)], axis=0)
    return out[perm]

